# revision 5
# baseline (speedup 1.0000x reference)
"""ComplexBatchNorm2d (Trabelsi-style complex whitening BN) on 8 trn2 NeuronCores.

Sharding: over channels C (8 channels per core). Each channel's batch statistics
are computed entirely on one core, so no collectives are needed.

The DMA engines are the roofline (67.1 MB/core at 360 GB/s = 186.6 us), so the
schedule keeps them saturated end to end (~99% busy, simulated 190.8 us):
  - channel data is pure interleaved [X(64)|Y(64)] chunks, loaded in quarter
    tiles on the SP queue; per-chunk gram matmuls (fp32r, 256-wide moving)
    accumulate the 2x2 Gram in PSUM while a second tiny matmul against a
    constant [ones|0] tile accumulates the column sums, so the loads carry
    zero layout overhead;
  - stats extraction is 3 masked mul+reduce ops (DVE) + 2 ACT copies; two
    masked fold matmuls (all-ones over the top/bottom 64 partitions)
    replicate the five sums across all partitions with no broadcast;
  - the 2x2 inverse-sqrt assembly runs on the otherwise-idle Pool engine
    (sqrts on ACT, one reciprocal on DVE); each consuming engine reads the
    resulting coefficients from a tile staged by a copy on its own engine,
    so program order guarantees readiness (no DRAM bounce);
  - whiten is split: ACT computes y = xr*G + B' via Identity activation
    (AP scale/bias), DVE adds xi*G' in place, per half-quarter block;
  - stores issue from the Pool (SWDGE) queue -- and for the last channels
    alternately from the then-idle SP queue -- so a store waiting on
    compute never stalls a load;
  - everything is software-pipelined with a 2-channel lag (iteration i
    emits loads/gram(i), extract(i-1), asm/whiten/stores(i-2), with
    folds(i-2) slotted between gram quarters) so no engine queue head
    ever blocks on a not-yet-ready wait.

Host side: slices/permutes inputs per core, builds the interleaved chunk
layout, gathers per-core outputs and permutes back to (B, C, H, W, 2).
"""

import numpy as np

# Problem geometry (hardcoded per contract).
B, C, H, W = 32, 64, 128, 128
NCORES = 8
CLOC = C // NCORES          # channels per core = 8
P = 128                     # SBUF partitions
N = B * H * W               # samples per channel = 524288
F = N // P                  # free columns per channel = 4096
CHUNK = 64                  # data columns per gram chunk
NCHUNK = F // CHUNK         # 64 chunks per channel
# [X(64) | Y(64)] per chunk -- pure data, no ones/pad columns. Column sums
# (for the means) come from a second accumulating matmul per chunk against
# a constant [P, 2] ones tile, so the loads carry zero layout overhead.
BLK = 2 * CHUNK             # 128 cols per chunk
XYW = NCHUNK * BLK          # 8192 free cols per channel (interleaved layout)
YW = 2 * F                  # 8192 output cols per channel (re/im interleaved)
EPS = 1e-5

NQ = 4                      # load quarters per channel
QCH = NCHUNK // NQ          # chunks per quarter = 16
QW = QCH * BLK              # 2048 cols per quarter tile
QOUT = QCH * 2 * CHUNK      # 2048 output cols per quarter
HCH = QCH // 2              # chunks per whiten block (half quarter) = 8
HOUT = HCH * 2 * CHUNK      # 1024 output cols per whiten block (re/im)

_CACHE = {}
_TRACE = False   # test.py sets this to capture NTFF profile / HW exec time
LAST = {}        # kernel() stores exec_time_ns etc. here

# tuning knobs
XY_BUFS = 4      # quarter tiles in flight per quarter-slot (ring per tag)
Y_BUFS = 12      # whiten output tiles in flight


def _build_nc():
    import concourse.bacc as bacc
    import concourse.mybir as mybir
    from concourse.tile import TileContext

    f32 = mybir.dt.float32
    f32r = mybir.dt.float32r
    Alu = mybir.AluOpType
    Act = mybir.ActivationFunctionType
    Axis = mybir.AxisListType

    nc = bacc.Bacc("TRN2", target_bir_lowering=False)
    # xy carries float32 bits but is declared float32r end-to-end so the BIR
    # verifier accepts it as a (fast-path) FP32r matmul operand.
    xy_d = nc.declare_dram_parameter("xy", [CLOC, P, XYW], f32r, isOutput=False)
    consts_d = nc.declare_dram_parameter("consts", [P, CHUNK], f32, isOutput=False)
    gb_d = nc.declare_dram_parameter("gb", [P, 48], f32, isOutput=False)
    y_d = nc.declare_dram_parameter("y", [CLOC, P, YW], f32, isOutput=True)

    V = nc.vector
    A = nc.scalar
    GS = nc.gpsimd

    rN = 1.0 / N
    rN1 = 1.0 / (N - 1)

    with TileContext(nc) as tc:
        with (
            tc.tile_pool(name="singles", bufs=1) as singles,
            tc.tile_pool(name="xyp", bufs=XY_BUFS) as xyp,
            tc.tile_pool(name="yp", bufs=Y_BUFS) as yp,
            tc.tile_pool(name="spp", bufs=2) as spp,
            tc.tile_pool(name="smallp", bufs=2) as smallp,
            tc.tile_pool(name="gramp", bufs=2, space="PSUM") as gramp,
            tc.tile_pool(name="g2p", bufs=2, space="PSUM") as g2p,
            tc.tile_pool(name="foldp", bufs=2, space="PSUM") as foldp,
        ):
            # consts/gb ride the ACT queue so the SP queue's first entry is
            # channel 0's first big load.
            consts = singles.tile([P, CHUNK], f32)
            A.dma_start(out=consts[:], in_=consts_d[:])
            gb = singles.tile([P, 48], f32)
            A.dma_start(out=gb[:], in_=gb_d[:])

            # DVE-staged identity so the masked-diag reduce ops are
            # same-engine after this copy.
            ident = singles.tile([P, CHUNK], f32)
            V.tensor_copy(ident[:], consts[:])

            # Partition-fold masks: mask_t sums partitions 0:64 into every
            # output partition, mask_b sums partitions 64:128.
            mask_t = singles.tile([P, P], f32)
            GS.memset(mask_t[0:CHUNK, :], 1.0)
            GS.memset(mask_t[CHUNK:P, :], 0.0)
            mask_b = singles.tile([P, P], f32)
            GS.memset(mask_b[0:CHUNK, :], 0.0)
            GS.memset(mask_b[CHUNK:P, :], 1.0)
            # [ones | zeros] moving operand for the per-chunk col-sum
            # matmuls (the zero column keeps the moving width even).
            ones2 = singles.tile([P, 2], f32)
            GS.memset(ones2[:, 0:1], 1.0)
            GS.memset(ones2[:, 1:2], 0.0)

            # Warm the ACT function table off the critical path: sqrt first
            # so the chosen set (sqrt_and_others) also covers Identity/Copy.
            warm = singles.tile([P, 1], f32)
            A.sqrt(warm[:], consts[:, 0:1])
            A.activation(warm[:], consts[:, 0:1], Act.Identity,
                         bias=consts[:, 0:1], scale=1.0)

            state = {}

            def emit_loads(c):
                xq = []
                for q in range(NQ):
                    xt = xyp.tile([P, QW], f32r, tag=f"xy{q}")
                    nc.sync.dma_start(
                        out=xt[:], in_=xy_d[c][:, q * QW:(q + 1) * QW])
                    xq.append(xt)
                return xq

            def emit_gram(c, xq):
                g = gramp.tile([P, 2 * BLK], f32, tag="g")
                g2 = g2p.tile([P, 2], f32, tag="g2")
                state[c]["g2"] = g2
                for q in range(NQ):
                    for j in range(QCH):
                        w = 2 * BLK if j < QCH - 1 else BLK
                        first = q == 0 and j == 0
                        last = q == NQ - 1 and j == QCH - 1
                        lhsT = xq[q][:, j * BLK: j * BLK + 2 * CHUNK]
                        nc.tensor.matmul(
                            g[:, 0:w], lhsT=lhsT,
                            rhs=xq[q][:, j * BLK: j * BLK + w],
                            start=first, stop=last,
                        )
                        # col sums: same stationary, [ones|0] moving ->
                        # g2 accumulates per-column sums.
                        nc.tensor.matmul(
                            g2[:, :], lhsT=lhsT,
                            rhs=ones2[:].bitcast(f32r),
                            start=first, stop=last,
                        )
                    if q == 0 and c >= 2:
                        # Channel c-2's partition folds slot in between this
                        # channel's gram quarters: their input (extract(c-2))
                        # ran a full channel-period ago, so the PE never
                        # blocks on them. A fold placed any earlier stalls
                        # the PE queue on the extract it depends on, and the
                        # whole gram stream behind it.
                        emit_folds(c - 2)
                return g

            def emit_extract(c, g):
                # ---- extract: sp col layout:
                #   col 0 = [sum x^2 partials ; sum y^2 partials]
                #   col 1 = [col-sums of x    ; col-sums of y    ]
                #   col 2 = [sum x*y partials ; (zeroed)         ]
                #   col 3 = zero pad (keeps fold moving width even)
                sp = spp.tile([P, 4], f32, tag="sp")
                GS.memset(sp[CHUNK:P, 2:3], 0.0)
                GS.memset(sp[:, 3:4], 0.0)
                junk = smallp.tile([P, CHUNK], f32, tag="junk")
                V.tensor_mul(junk[0:CHUNK, :], g[0:CHUNK, 0:CHUNK],
                             ident[0:CHUNK, :])
                V.tensor_reduce(out=sp[0:CHUNK, 0:1], in_=junk[0:CHUNK, :],
                                axis=Axis.X, op=Alu.add)
                V.tensor_mul(junk[CHUNK:P, :], g[CHUNK:P, CHUNK:2 * CHUNK],
                             ident[CHUNK:P, :])
                V.tensor_reduce(out=sp[CHUNK:P, 0:1], in_=junk[CHUNK:P, :],
                                axis=Axis.X, op=Alu.add)
                V.tensor_mul(junk[0:CHUNK, :], g[0:CHUNK, CHUNK:2 * CHUNK],
                             ident[0:CHUNK, :])
                V.tensor_reduce(out=sp[0:CHUNK, 2:3], in_=junk[0:CHUNK, :],
                                axis=Axis.X, op=Alu.add)
                g2 = state[c]["g2"]
                A.copy(sp[0:CHUNK, 1:2], g2[0:CHUNK, 0:1])
                A.copy(sp[CHUNK:P, 1:2], g2[CHUNK:P, 0:1])
                return sp

            def emit_folds(c):
                sp = state[c]["sp"]
                sfold = foldp.tile([P, 8], f32, tag="f")
                nc.tensor.matmul(sfold[:, 0:4], lhsT=mask_t[:], rhs=sp[:, :],
                                 start=True, stop=True)
                nc.tensor.matmul(sfold[:, 4:8], lhsT=mask_b[:], rhs=sp[:, :],
                                 start=True, stop=True)
                state[c]["sfold"] = sfold

            def emit_asm(c):
                sfold = state[c]["sfold"]
                # ACT stages the fold PSUM into SBUF so the Pool-engine
                # assembly never touches PSUM.
                ssb = smallp.tile([P, 8], f32, tag="ssb")
                A.copy(ssb[:], sfold[:])
                SXX, SR, SXY = ssb[:, 0:1], ssb[:, 1:2], ssb[:, 2:3]
                SYY, SI = ssb[:, 4:5], ssb[:, 5:6]

                # ---- 2x2 assembly on the (otherwise idle) Pool engine,
                # replicated across partitions; sqrts on ACT ----
                tmp = smallp.tile([P, 16], f32, tag="tmp")

                def ts(i, tmp=tmp):
                    return tmp[:, i:i + 1]

                MR, MI, u = ts(0), ts(1), ts(2)
                a, bb, cc = ts(3), ts(4), ts(5)
                GS.tensor_scalar_mul(MR, SR, rN)
                GS.tensor_scalar_mul(MI, SI, rN)
                GS.tensor_mul(u, SR, MR)
                GS.tensor_sub(a, SXX, u)
                GS.tensor_scalar(out=a, in0=a, scalar1=rN1, scalar2=EPS,
                                 op0=Alu.mult, op1=Alu.add)
                GS.tensor_mul(u, SR, MI)
                GS.tensor_sub(bb, SXY, u)
                GS.tensor_scalar_mul(bb, bb, rN1)
                GS.tensor_mul(u, SI, MI)
                GS.tensor_sub(cc, SYY, u)
                GS.tensor_scalar(out=cc, in0=cc, scalar1=rN1, scalar2=EPS,
                                 op0=Alu.mult, op1=Alu.add)
                # (M)^{-1/2} for M=[[a,b],[b,c]]: s=sqrt(ac-b^2);
                # t=sqrt(a+c+2s); W=[[c+s,-b],[-b,a+s]]/(s*t)
                det, tr, tr2, st = ts(6), ts(7), ts(8), ts(9)
                GS.tensor_mul(det, a, cc)
                GS.tensor_mul(u, bb, bb)
                GS.tensor_sub(det, det, u)
                GS.tensor_add(tr, a, cc)
                sA = smallp.tile([P, 1], f32, tag="sA")
                tA = smallp.tile([P, 1], f32, tag="tA")
                A.sqrt(sA[:], det)
                GS.tensor_add(u, sA[:], sA[:])
                GS.tensor_add(tr2, u, tr)
                A.sqrt(tA[:], tr2)
                GS.tensor_mul(st, sA[:], tA[:])
                # 1/(s*t) on DVE (Pool has no divide ALU); the one-op hop
                # sits between extract and whiten stage-2 in the DVE stream.
                inv = smallp.tile([P, 1], f32, tag="inv")
                V.reciprocal(inv[:], st)
                w00, w01, w11, q1, q2 = ts(10), ts(11), ts(12), ts(13), ts(14)
                GS.tensor_add(w00, cc, sA[:])
                GS.tensor_mul(w00, w00, inv[:])
                GS.tensor_mul(w01, bb, inv[:])
                GS.tensor_scalar_mul(w01, w01, -1.0)
                GS.tensor_add(w11, a, sA[:])
                GS.tensor_mul(w11, w11, inv[:])
                # G = gamma @ W ; B' = beta - G @ mean
                g00 = gb[:, 0 * 8 + c: 0 * 8 + c + 1]
                g01 = gb[:, 1 * 8 + c: 1 * 8 + c + 1]
                g10 = gb[:, 2 * 8 + c: 2 * 8 + c + 1]
                g11 = gb[:, 3 * 8 + c: 3 * 8 + c + 1]
                br_ = gb[:, 4 * 8 + c: 4 * 8 + c + 1]
                bi_ = gb[:, 5 * 8 + c: 5 * 8 + c + 1]
                cb = smallp.tile([P, 6], f32, tag="cb")
                G00, G01, BR = cb[:, 0:1], cb[:, 1:2], cb[:, 2:3]
                G10, G11, BI = cb[:, 3:4], cb[:, 4:5], cb[:, 5:6]
                GS.tensor_mul(q1, g00, w00)
                GS.tensor_mul(q2, g01, w01)
                GS.tensor_add(G00, q1, q2)
                GS.tensor_mul(q1, g00, w01)
                GS.tensor_mul(q2, g01, w11)
                GS.tensor_add(G01, q1, q2)
                GS.tensor_mul(q1, g10, w00)
                GS.tensor_mul(q2, g11, w01)
                GS.tensor_add(G10, q1, q2)
                GS.tensor_mul(q1, g10, w01)
                GS.tensor_mul(q2, g11, w11)
                GS.tensor_add(G11, q1, q2)
                GS.tensor_mul(q1, G00, MR)
                GS.tensor_mul(q2, G01, MI)
                GS.tensor_add(q1, q1, q2)
                GS.tensor_sub(BR, br_, q1)
                GS.tensor_mul(q1, G10, MR)
                GS.tensor_mul(q2, G11, MI)
                GS.tensor_add(q1, q1, q2)
                GS.tensor_sub(BI, bi_, q1)
                # Per-engine staged copies: whiten stage-1 (ACT) and stage-2
                # (DVE) read scale/bias scalars from tiles produced on their
                # own engine, so program order guarantees readiness.
                cbA = smallp.tile([P, 6], f32, tag="cbA")
                A.copy(cbA[:], cb[:])
                cbD = smallp.tile([P, 6], f32, tag="cbD")
                V.tensor_copy(cbD[:], cb[:])
                state[c]["cb"] = cb
                state[c]["cbA"] = cbA
                state[c]["cbD"] = cbD

            def emit_whiten(c):
                # ---- whiten + affine + store, per half-quarter block ----
                xq = state[c]["xq"]
                cbA, cbD = state[c]["cbA"], state[c]["cbD"]
                for h in range(2 * NQ):
                    q, hh = h // 2, h % 2
                    x3 = xq[q][:].bitcast(f32).rearrange(
                        "p (j k) -> p j k", k=BLK)
                    xr = x3[:, hh * HCH:(hh + 1) * HCH, 0:CHUNK]
                    xi = x3[:, hh * HCH:(hh + 1) * HCH, CHUNK:2 * CHUNK]
                    yt = yp.tile([P, HCH, 2 * CHUNK], f32, tag="y")
                    ye = yt[:, :, 0:2 * CHUNK:2]
                    yo = yt[:, :, 1:2 * CHUNK:2]
                    A.activation(ye, xr, Act.Identity,
                                 bias=cbA[:, 2:3], scale=cbA[:, 0:1])
                    A.activation(yo, xr, Act.Identity,
                                 bias=cbA[:, 5:6], scale=cbA[:, 3:4])
                    V.scalar_tensor_tensor(out=ye, in0=xi,
                                           scalar=cbD[:, 1:2],
                                           in1=ye, op0=Alu.mult, op1=Alu.add)
                    V.scalar_tensor_tensor(out=yo, in0=xi,
                                           scalar=cbD[:, 4:5],
                                           in1=yo, op0=Alu.mult, op1=Alu.add)
                    # Late channels run after the last load has issued, so
                    # the idle SP queue takes every other store; Pool's
                    # SWDGE generation alone would pace the drain.
                    eng = nc.sync if (c >= CLOC - 3 and h % 2 == 1) else GS
                    eng.dma_start(
                        out=y_d[c][:, h * HOUT:(h + 1) * HOUT],
                        in_=yt[:].rearrange("p a b -> p (a b)"))

            # Lag-2 software pipeline. Per iteration i the engine streams see:
            #   SP:   loads(i)
            #   PE:   gram(i) [folds(i-2) slotted in after quarter 0]
            #   DVE:  extract(i-1), asm(i-2), whiten-stage2(i-2)
            #   ACT:  colsum-copies(i-1), sqrts/cbA(i-2), whiten-stage1(i-2)
            #   Pool: sp-memsets(i-1), stores(i-2)
            # Every queue head's semaphore wait targets work from >= one
            # channel-period earlier, so no engine ever idles with ready
            # work parked behind a blocked instruction.
            for i in range(CLOC + 2):
                if i < CLOC:
                    state[i] = {}
                    state[i]["xq"] = emit_loads(i)
                    state[i]["g"] = emit_gram(i, state[i]["xq"])
                else:
                    emit_folds(i - 2)
                if 1 <= i <= CLOC:
                    state[i - 1]["sp"] = emit_extract(i - 1,
                                                      state[i - 1]["g"])
                if i >= 2:
                    emit_asm(i - 2)
                    emit_whiten(i - 2)

    nc.finalize()
    return nc


def _get_nc():
    if "nc" not in _CACHE:
        _CACHE["nc"] = _build_nc()
    return _CACHE["nc"]


def _prep_consts():
    ident = np.zeros((P, CHUNK), np.float32)
    ident[np.arange(P), np.arange(P) % CHUNK] = 1.0
    return ident


def _prep_core(x_real, x_imag, gamma, beta, k):
    c0 = k * CLOC
    xr = np.ascontiguousarray(
        x_real[:, c0:c0 + CLOC].transpose(1, 0, 2, 3)
    ).reshape(CLOC, P, NCHUNK, CHUNK)
    xi = np.ascontiguousarray(
        x_imag[:, c0:c0 + CLOC].transpose(1, 0, 2, 3)
    ).reshape(CLOC, P, NCHUNK, CHUNK)
    xy = np.empty((CLOC, P, NCHUNK, BLK), np.float32)
    xy[..., 0:CHUNK] = xr
    xy[..., CHUNK:2 * CHUNK] = xi
    g = gamma[c0:c0 + CLOC]
    b = beta[c0:c0 + CLOC]
    gb = np.concatenate([g[:, 0, 0], g[:, 0, 1], g[:, 1, 0], g[:, 1, 1],
                         b[:, 0], b[:, 1]]).astype(np.float32).reshape(1, 48)
    gb = np.broadcast_to(gb, (P, 48)).copy()
    return {"xy": xy.reshape(CLOC, P, XYW), "consts": _prep_consts(), "gb": gb}


def kernel(x_real, x_imag, gamma, beta):
    from concourse.bass_utils import run_bass_kernel_spmd

    x_real = np.asarray(x_real, dtype=np.float32)
    x_imag = np.asarray(x_imag, dtype=np.float32)
    gamma = np.asarray(gamma, dtype=np.float32)
    beta = np.asarray(beta, dtype=np.float32)

    in_maps = [_prep_core(x_real, x_imag, gamma, beta, k)
               for k in range(NCORES)]

    nc = _get_nc()
    res = None
    if _TRACE:
        try:
            res = run_bass_kernel_spmd(nc, in_maps, list(range(NCORES)),
                                       trace=True)
        except Exception as e:  # trace infra unavailable -> plain run
            LAST["trace_error"] = repr(e)
            res = None
    if res is None:
        res = run_bass_kernel_spmd(nc, in_maps, list(range(NCORES)))
    LAST["exec_time_ns"] = res.exec_time_ns
    LAST["mean_exec_time_ns"] = res.mean_exec_time_ns
    LAST["profile_json"] = res.profile_json

    out = np.empty((B, C, H, W, 2), np.float32)
    for k in range(NCORES):
        c0 = k * CLOC
        y = res.results[k]["y"].reshape(CLOC, N, 2).reshape(CLOC, B, H, W, 2)
        out[:, c0:c0 + CLOC] = y.transpose(1, 0, 2, 3, 4)
    return out


# revision 6
# speedup vs baseline: 1.0004x; 1.0004x over previous
"""ComplexBatchNorm2d (Trabelsi-style complex whitening BN) on 8 trn2 NeuronCores.

Sharding: over channels C (8 channels per core). Each channel's batch statistics
are computed entirely on one core, so no collectives are needed.

v2 schedule (vs v1): the DMA engines are the roofline (67.6 MB/core at
360 GB/s = 188.7 us), so every other engine is organized to never make a
DMA wait:
  - channel data is loaded in QUARTER tiles (4 DMAs per channel, SP queue)
    so the gram matmuls start ~9 us into each load instead of after it;
  - y stores are issued from the Pool (SWDGE) queue -- and for the last
    channels alternately from the then-idle SP queue -- so a store waiting
    on compute can never stall a load;
  - stats extraction is 3 masked mul+reduce pairs (DVE) + 2 ACT copies;
    two masked fold matmuls replace the per-channel ones-matmul+memset;
  - the 2x2 assembly runs on the otherwise-idle Pool engine (sqrts on
    ACT, one reciprocal on DVE), lag-2 software-pipelined so no engine
    queue ever blocks on a not-ready wait;
  - no DRAM bounce for the coefficients: DVE consumers read the DVE-written
    cb tile (same-engine program order), ACT consumers read an ACT-staged
    copy;
  - whiten is split: ACT computes y = xr*G + B' via Identity activation
    (AP scale/bias), DVE adds xi*G' in place, halving DVE work.

Host side: slices/permutes inputs per core, builds the interleaved chunk
layout, gathers per-core outputs and permutes back to (B, C, H, W, 2).
"""

import numpy as np

# Problem geometry (hardcoded per contract).
B, C, H, W = 32, 64, 128, 128
NCORES = 8
CLOC = C // NCORES          # channels per core = 8
P = 128                     # SBUF partitions
N = B * H * W               # samples per channel = 524288
F = N // P                  # free columns per channel = 4096
CHUNK = 64                  # data columns per gram chunk
NCHUNK = F // CHUNK         # 64 chunks per channel
# [X(64) | Y(64)] per chunk -- pure data, no ones/pad columns. Column sums
# (for the means) come from a second accumulating matmul per chunk against
# a constant [P, 2] ones tile, so the loads carry zero layout overhead.
BLK = 2 * CHUNK             # 128 cols per chunk
XYW = NCHUNK * BLK          # 8192 free cols per channel (interleaved layout)
YW = 2 * F                  # 8192 output cols per channel (re/im interleaved)
EPS = 1e-5

NQ = 4                      # load quarters per channel
QCH = NCHUNK // NQ          # chunks per quarter = 16
QW = QCH * BLK              # 2048 cols per quarter tile
QOUT = QCH * 2 * CHUNK      # 2048 output cols per quarter
HCH = QCH // 2              # chunks per whiten block (half quarter) = 8
HOUT = HCH * 2 * CHUNK      # 1024 output cols per whiten block

_CACHE = {}
_TRACE = False   # test.py sets this to capture NTFF profile / HW exec time
LAST = {}        # kernel() stores exec_time_ns etc. here

# tuning knobs
XY_BUFS = 4      # quarter tiles in flight per quarter-slot (ring per tag)
Y_BUFS = 12      # whiten output tiles in flight


def _build_nc():
    import concourse.bacc as bacc
    import concourse.mybir as mybir
    from concourse.tile import TileContext

    f32 = mybir.dt.float32
    f32r = mybir.dt.float32r
    Alu = mybir.AluOpType
    Act = mybir.ActivationFunctionType
    Axis = mybir.AxisListType

    nc = bacc.Bacc("TRN2", target_bir_lowering=False)
    # xy carries float32 bits but is declared float32r end-to-end so the BIR
    # verifier accepts it as a (fast-path) FP32r matmul operand.
    xy_d = nc.declare_dram_parameter("xy", [CLOC, P, XYW], f32r, isOutput=False)
    gb_d = nc.declare_dram_parameter("gb", [P, 48], f32, isOutput=False)
    y_d = nc.declare_dram_parameter("y", [CLOC, P, YW], f32, isOutput=True)

    V = nc.vector
    A = nc.scalar
    GS = nc.gpsimd

    rN = 1.0 / N
    rN1 = 1.0 / (N - 1)

    with TileContext(nc) as tc:
        with (
            tc.tile_pool(name="singles", bufs=1) as singles,
            tc.tile_pool(name="xyp", bufs=XY_BUFS) as xyp,
            tc.tile_pool(name="yp", bufs=Y_BUFS) as yp,
            tc.tile_pool(name="spp", bufs=2) as spp,
            tc.tile_pool(name="smallp", bufs=2) as smallp,
            tc.tile_pool(name="gramp", bufs=2, space="PSUM") as gramp,
            tc.tile_pool(name="g2p", bufs=2, space="PSUM") as g2p,
            tc.tile_pool(name="foldp", bufs=2, space="PSUM") as foldp,
        ):
            # gb rides the ACT queue so the SP queue's first entry is
            # channel 0's first big load.
            gb = singles.tile([P, 48], f32)
            A.dma_start(out=gb[:], in_=gb_d[:])

            # Partition-fold masks: mask_t sums partitions 0:64 into every
            # output partition, mask_b sums partitions 64:128.
            mask_t = singles.tile([P, P], f32)
            GS.memset(mask_t[0:CHUNK, :], 1.0)
            GS.memset(mask_t[CHUNK:P, :], 0.0)
            mask_b = singles.tile([P, P], f32)
            GS.memset(mask_b[0:CHUNK, :], 0.0)
            GS.memset(mask_b[CHUNK:P, :], 1.0)

            # Identity mask (ident[p, j] = 1 iff j == p mod 64), built
            # on-chip on Pool: two full-height affine diagonal selects
            # (predicates j - p == 0 and j - p + 64 == 0) summed. Full-height
            # APs only, so the result does not depend on whether the affine
            # channel index is absolute or AP-relative (a known sim/HW
            # divergence risk). Saves the constant-table DMA entirely.
            ones64 = singles.tile([P, CHUNK], f32)
            GS.memset(ones64[:], 1.0)
            diag_t = singles.tile([P, CHUNK], f32)
            GS.affine_select(diag_t[:], ones64[:],
                             pattern=[[1, CHUNK]], compare_op=Alu.is_equal,
                             fill=0.0, base=0, channel_multiplier=-1)
            ident = singles.tile([P, CHUNK], f32)
            GS.affine_select(ident[:], ones64[:],
                             pattern=[[1, CHUNK]], compare_op=Alu.is_equal,
                             fill=0.0, base=CHUNK, channel_multiplier=-1)
            GS.tensor_add(ident[:], ident[:], diag_t[:])
            # [ones | zeros] moving operand for the per-chunk col-sum
            # matmuls (the zero column keeps the moving width even).
            ones2 = singles.tile([P, 2], f32)
            GS.memset(ones2[:, 0:1], 1.0)
            GS.memset(ones2[:, 1:2], 0.0)

            # Warm the ACT function table off the critical path: sqrt first
            # so the chosen set (sqrt_and_others) also covers Identity/Copy.
            warm = singles.tile([P, 1], f32)
            A.sqrt(warm[:], gb[:, 0:1])
            A.activation(warm[:], gb[:, 0:1], Act.Identity,
                         bias=gb[:, 0:1], scale=1.0)

            state = {}

            def emit_loads(c):
                xq = []
                for q in range(NQ):
                    xt = xyp.tile([P, QW], f32r, tag=f"xy{q}")
                    nc.sync.dma_start(
                        out=xt[:], in_=xy_d[c][:, q * QW:(q + 1) * QW])
                    xq.append(xt)
                return xq

            def emit_gram(c, xq):
                g = gramp.tile([P, 2 * BLK], f32, tag="g")
                g2 = g2p.tile([P, 2], f32, tag="g2")
                state[c]["g2"] = g2
                for q in range(NQ):
                    for j in range(QCH):
                        w = 2 * BLK if j < QCH - 1 else BLK
                        first = q == 0 and j == 0
                        last = q == NQ - 1 and j == QCH - 1
                        lhsT = xq[q][:, j * BLK: j * BLK + 2 * CHUNK]
                        nc.tensor.matmul(
                            g[:, 0:w], lhsT=lhsT,
                            rhs=xq[q][:, j * BLK: j * BLK + w],
                            start=first, stop=last,
                        )
                        # col sums: same stationary, [ones|0] moving ->
                        # g2 accumulates per-column sums.
                        nc.tensor.matmul(
                            g2[:, :], lhsT=lhsT,
                            rhs=ones2[:].bitcast(f32r),
                            start=first, stop=last,
                        )
                    if q == 0 and c >= 2:
                        # Channel c-2's partition folds slot in between this
                        # channel's gram quarters: their input (extract(c-2))
                        # ran a full channel-period ago, so the PE never
                        # blocks on them. A fold placed any earlier stalls
                        # the PE queue on the extract it depends on, and the
                        # whole gram stream behind it.
                        emit_folds(c - 2)
                return g

            def emit_extract(c, g):
                # ---- extract: sp col layout:
                #   col 0 = [sum x^2 partials ; sum y^2 partials]
                #   col 1 = [col-sums of x    ; col-sums of y    ]
                #   col 2 = [sum x*y partials ; (zeroed)         ]
                #   col 3 = zero pad (keeps fold moving width even)
                sp = spp.tile([P, 4], f32, tag="sp")
                GS.memset(sp[CHUNK:P, 2:3], 0.0)
                GS.memset(sp[:, 3:4], 0.0)
                junk = smallp.tile([P, CHUNK], f32, tag="junk")
                V.tensor_mul(junk[0:CHUNK, :], g[0:CHUNK, 0:CHUNK],
                             ident[0:CHUNK, :])
                V.tensor_reduce(out=sp[0:CHUNK, 0:1], in_=junk[0:CHUNK, :],
                                axis=Axis.X, op=Alu.add)
                V.tensor_mul(junk[CHUNK:P, :], g[CHUNK:P, CHUNK:2 * CHUNK],
                             ident[CHUNK:P, :])
                V.tensor_reduce(out=sp[CHUNK:P, 0:1], in_=junk[CHUNK:P, :],
                                axis=Axis.X, op=Alu.add)
                V.tensor_mul(junk[0:CHUNK, :], g[0:CHUNK, CHUNK:2 * CHUNK],
                             ident[0:CHUNK, :])
                V.tensor_reduce(out=sp[0:CHUNK, 2:3], in_=junk[0:CHUNK, :],
                                axis=Axis.X, op=Alu.add)
                g2 = state[c]["g2"]
                A.copy(sp[0:CHUNK, 1:2], g2[0:CHUNK, 0:1])
                A.copy(sp[CHUNK:P, 1:2], g2[CHUNK:P, 0:1])
                return sp

            def emit_folds(c):
                sp = state[c]["sp"]
                sfold = foldp.tile([P, 8], f32, tag="f")
                nc.tensor.matmul(sfold[:, 0:4], lhsT=mask_t[:], rhs=sp[:, :],
                                 start=True, stop=True)
                nc.tensor.matmul(sfold[:, 4:8], lhsT=mask_b[:], rhs=sp[:, :],
                                 start=True, stop=True)
                state[c]["sfold"] = sfold

            def emit_asm(c):
                sfold = state[c]["sfold"]
                # ACT stages the fold PSUM into SBUF so the Pool-engine
                # assembly never touches PSUM.
                ssb = smallp.tile([P, 8], f32, tag="ssb")
                A.copy(ssb[:], sfold[:])
                SXX, SR, SXY = ssb[:, 0:1], ssb[:, 1:2], ssb[:, 2:3]
                SYY, SI = ssb[:, 4:5], ssb[:, 5:6]

                # ---- 2x2 assembly on the (otherwise idle) Pool engine,
                # replicated across partitions; sqrts on ACT ----
                tmp = smallp.tile([P, 16], f32, tag="tmp")

                def ts(i, tmp=tmp):
                    return tmp[:, i:i + 1]

                MR, MI, u = ts(0), ts(1), ts(2)
                a, bb, cc = ts(3), ts(4), ts(5)
                GS.tensor_scalar_mul(MR, SR, rN)
                GS.tensor_scalar_mul(MI, SI, rN)
                GS.tensor_mul(u, SR, MR)
                GS.tensor_sub(a, SXX, u)
                GS.tensor_scalar(out=a, in0=a, scalar1=rN1, scalar2=EPS,
                                 op0=Alu.mult, op1=Alu.add)
                GS.tensor_mul(u, SR, MI)
                GS.tensor_sub(bb, SXY, u)
                GS.tensor_scalar_mul(bb, bb, rN1)
                GS.tensor_mul(u, SI, MI)
                GS.tensor_sub(cc, SYY, u)
                GS.tensor_scalar(out=cc, in0=cc, scalar1=rN1, scalar2=EPS,
                                 op0=Alu.mult, op1=Alu.add)
                # (M)^{-1/2} for M=[[a,b],[b,c]]: s=sqrt(ac-b^2);
                # t=sqrt(a+c+2s); W=[[c+s,-b],[-b,a+s]]/(s*t)
                det, tr, tr2, st = ts(6), ts(7), ts(8), ts(9)
                GS.tensor_mul(det, a, cc)
                GS.tensor_mul(u, bb, bb)
                GS.tensor_sub(det, det, u)
                GS.tensor_add(tr, a, cc)
                sA = smallp.tile([P, 1], f32, tag="sA")
                tA = smallp.tile([P, 1], f32, tag="tA")
                A.sqrt(sA[:], det)
                GS.tensor_add(u, sA[:], sA[:])
                GS.tensor_add(tr2, u, tr)
                A.sqrt(tA[:], tr2)
                GS.tensor_mul(st, sA[:], tA[:])
                # 1/(s*t) on DVE (Pool has no divide ALU); the one-op hop
                # sits between extract and whiten stage-2 in the DVE stream.
                inv = smallp.tile([P, 1], f32, tag="inv")
                V.reciprocal(inv[:], st)
                w00, w01, w11, q1, q2 = ts(10), ts(11), ts(12), ts(13), ts(14)
                GS.tensor_add(w00, cc, sA[:])
                GS.tensor_mul(w00, w00, inv[:])
                GS.tensor_mul(w01, bb, inv[:])
                GS.tensor_scalar_mul(w01, w01, -1.0)
                GS.tensor_add(w11, a, sA[:])
                GS.tensor_mul(w11, w11, inv[:])
                # G = gamma @ W ; B' = beta - G @ mean
                g00 = gb[:, 0 * 8 + c: 0 * 8 + c + 1]
                g01 = gb[:, 1 * 8 + c: 1 * 8 + c + 1]
                g10 = gb[:, 2 * 8 + c: 2 * 8 + c + 1]
                g11 = gb[:, 3 * 8 + c: 3 * 8 + c + 1]
                br_ = gb[:, 4 * 8 + c: 4 * 8 + c + 1]
                bi_ = gb[:, 5 * 8 + c: 5 * 8 + c + 1]
                cb = smallp.tile([P, 6], f32, tag="cb")
                G00, G01, BR = cb[:, 0:1], cb[:, 1:2], cb[:, 2:3]
                G10, G11, BI = cb[:, 3:4], cb[:, 4:5], cb[:, 5:6]
                GS.tensor_mul(q1, g00, w00)
                GS.tensor_mul(q2, g01, w01)
                GS.tensor_add(G00, q1, q2)
                GS.tensor_mul(q1, g00, w01)
                GS.tensor_mul(q2, g01, w11)
                GS.tensor_add(G01, q1, q2)
                GS.tensor_mul(q1, g10, w00)
                GS.tensor_mul(q2, g11, w01)
                GS.tensor_add(G10, q1, q2)
                GS.tensor_mul(q1, g10, w01)
                GS.tensor_mul(q2, g11, w11)
                GS.tensor_add(G11, q1, q2)
                GS.tensor_mul(q1, G00, MR)
                GS.tensor_mul(q2, G01, MI)
                GS.tensor_add(q1, q1, q2)
                GS.tensor_sub(BR, br_, q1)
                GS.tensor_mul(q1, G10, MR)
                GS.tensor_mul(q2, G11, MI)
                GS.tensor_add(q1, q1, q2)
                GS.tensor_sub(BI, bi_, q1)
                # Per-engine staged copies: whiten stage-1 (ACT) and stage-2
                # (DVE) read scale/bias scalars from tiles produced on their
                # own engine, so program order guarantees readiness.
                cbA = smallp.tile([P, 6], f32, tag="cbA")
                A.copy(cbA[:], cb[:])
                cbD = smallp.tile([P, 6], f32, tag="cbD")
                V.tensor_copy(cbD[:], cb[:])
                state[c]["cb"] = cb
                state[c]["cbA"] = cbA
                state[c]["cbD"] = cbD

            def emit_whiten(c):
                # ---- whiten + affine + store, per half-quarter block ----
                xq = state[c]["xq"]
                cbA, cbD = state[c]["cbA"], state[c]["cbD"]
                for h in range(2 * NQ):
                    q, hh = h // 2, h % 2
                    x3 = xq[q][:].bitcast(f32).rearrange(
                        "p (j k) -> p j k", k=BLK)
                    xr = x3[:, hh * HCH:(hh + 1) * HCH, 0:CHUNK]
                    xi = x3[:, hh * HCH:(hh + 1) * HCH, CHUNK:2 * CHUNK]
                    yt = yp.tile([P, HCH, 2 * CHUNK], f32, tag="y")
                    ye = yt[:, :, 0:2 * CHUNK:2]
                    yo = yt[:, :, 1:2 * CHUNK:2]
                    A.activation(ye, xr, Act.Identity,
                                 bias=cbA[:, 2:3], scale=cbA[:, 0:1])
                    A.activation(yo, xr, Act.Identity,
                                 bias=cbA[:, 5:6], scale=cbA[:, 3:4])
                    V.scalar_tensor_tensor(out=ye, in0=xi,
                                           scalar=cbD[:, 1:2],
                                           in1=ye, op0=Alu.mult, op1=Alu.add)
                    V.scalar_tensor_tensor(out=yo, in0=xi,
                                           scalar=cbD[:, 4:5],
                                           in1=yo, op0=Alu.mult, op1=Alu.add)
                    # Late channels run after the last load has issued, so
                    # the idle SP queue takes every other store; Pool's
                    # SWDGE generation alone would pace the drain.
                    eng = nc.sync if (c >= CLOC - 3 and h % 2 == 1) else GS
                    eng.dma_start(
                        out=y_d[c][:, h * HOUT:(h + 1) * HOUT],
                        in_=yt[:].rearrange("p a b -> p (a b)"))

            # Lag-2 software pipeline. Per iteration i the engine streams see:
            #   SP:   loads(i)
            #   PE:   gram(i) [folds(i-2) slotted in after quarter 0]
            #   DVE:  extract(i-1), asm(i-2), whiten-stage2(i-2)
            #   ACT:  colsum-copies(i-1), sqrts/cbA(i-2), whiten-stage1(i-2)
            #   Pool: sp-memsets(i-1), stores(i-2)
            # Every queue head's semaphore wait targets work from >= one
            # channel-period earlier, so no engine ever idles with ready
            # work parked behind a blocked instruction.
            for i in range(CLOC + 2):
                if i < CLOC:
                    state[i] = {}
                    state[i]["xq"] = emit_loads(i)
                    state[i]["g"] = emit_gram(i, state[i]["xq"])
                else:
                    emit_folds(i - 2)
                if 1 <= i <= CLOC:
                    state[i - 1]["sp"] = emit_extract(i - 1,
                                                      state[i - 1]["g"])
                if i >= 2:
                    emit_asm(i - 2)
                    emit_whiten(i - 2)

    nc.finalize()
    return nc


def _get_nc():
    if "nc" not in _CACHE:
        _CACHE["nc"] = _build_nc()
    return _CACHE["nc"]


def _prep_core(x_real, x_imag, gamma, beta, k):
    c0 = k * CLOC
    xr = np.ascontiguousarray(
        x_real[:, c0:c0 + CLOC].transpose(1, 0, 2, 3)
    ).reshape(CLOC, P, NCHUNK, CHUNK)
    xi = np.ascontiguousarray(
        x_imag[:, c0:c0 + CLOC].transpose(1, 0, 2, 3)
    ).reshape(CLOC, P, NCHUNK, CHUNK)
    xy = np.empty((CLOC, P, NCHUNK, BLK), np.float32)
    xy[..., 0:CHUNK] = xr
    xy[..., CHUNK:2 * CHUNK] = xi
    g = gamma[c0:c0 + CLOC]
    b = beta[c0:c0 + CLOC]
    gb = np.concatenate([g[:, 0, 0], g[:, 0, 1], g[:, 1, 0], g[:, 1, 1],
                         b[:, 0], b[:, 1]]).astype(np.float32).reshape(1, 48)
    gb = np.broadcast_to(gb, (P, 48)).copy()
    return {"xy": xy.reshape(CLOC, P, XYW), "gb": gb}


def kernel(x_real, x_imag, gamma, beta):
    from concourse.bass_utils import run_bass_kernel_spmd

    x_real = np.asarray(x_real, dtype=np.float32)
    x_imag = np.asarray(x_imag, dtype=np.float32)
    gamma = np.asarray(gamma, dtype=np.float32)
    beta = np.asarray(beta, dtype=np.float32)

    in_maps = [_prep_core(x_real, x_imag, gamma, beta, k)
               for k in range(NCORES)]

    nc = _get_nc()
    res = None
    if _TRACE:
        try:
            res = run_bass_kernel_spmd(nc, in_maps, list(range(NCORES)),
                                       trace=True)
        except Exception as e:  # trace infra unavailable -> plain run
            LAST["trace_error"] = repr(e)
            res = None
    if res is None:
        res = run_bass_kernel_spmd(nc, in_maps, list(range(NCORES)))
    LAST["exec_time_ns"] = res.exec_time_ns
    LAST["mean_exec_time_ns"] = res.mean_exec_time_ns
    LAST["profile_json"] = res.profile_json

    out = np.empty((B, C, H, W, 2), np.float32)
    for k in range(NCORES):
        c0 = k * CLOC
        y = res.results[k]["y"].reshape(CLOC, N, 2).reshape(CLOC, B, H, W, 2)
        out[:, c0:c0 + CLOC] = y.transpose(1, 0, 2, 3, 4)
    return out


# revision 7
# speedup vs baseline: 1.0035x; 1.0031x over previous
"""ComplexBatchNorm2d (Trabelsi-style complex whitening BN) on 8 trn2 NeuronCores.

Sharding: over channels C (8 channels per core). Each channel's batch statistics
are computed entirely on one core, so no collectives are needed.

v2 schedule (vs v1): the DMA engines are the roofline (67.6 MB/core at
360 GB/s = 188.7 us), so every other engine is organized to never make a
DMA wait:
  - channel data is loaded in QUARTER tiles (4 DMAs per channel, SP queue)
    so the gram matmuls start ~9 us into each load instead of after it;
  - y stores are issued from the Pool (SWDGE) queue -- and for the last
    channels alternately from the then-idle SP queue -- so a store waiting
    on compute can never stall a load;
  - stats extraction is 3 masked mul+reduce pairs (DVE) + 2 ACT copies;
    two masked fold matmuls replace the per-channel ones-matmul+memset;
  - the 2x2 assembly runs on the otherwise-idle Pool engine (sqrts on
    ACT, one reciprocal on DVE), lag-2 software-pipelined so no engine
    queue ever blocks on a not-ready wait;
  - no DRAM bounce for the coefficients: DVE consumers read the DVE-written
    cb tile (same-engine program order), ACT consumers read an ACT-staged
    copy;
  - whiten is split: ACT computes y = xr*G + B' via Identity activation
    (AP scale/bias), DVE adds xi*G' in place, halving DVE work.

Host side: slices/permutes inputs per core, builds the interleaved chunk
layout, gathers per-core outputs and permutes back to (B, C, H, W, 2).
"""

import numpy as np

# Problem geometry (hardcoded per contract).
B, C, H, W = 32, 64, 128, 128
NCORES = 8
CLOC = C // NCORES          # channels per core = 8
P = 128                     # SBUF partitions
N = B * H * W               # samples per channel = 524288
F = N // P                  # free columns per channel = 4096
CHUNK = 64                  # data columns per gram chunk
NCHUNK = F // CHUNK         # 64 chunks per channel
# [X(64) | Y(64)] per chunk -- pure data, no ones/pad columns. Column sums
# (for the means) come from a second accumulating matmul per chunk against
# a constant [P, 2] ones tile, so the loads carry zero layout overhead.
BLK = 2 * CHUNK             # 128 cols per chunk
XYW = NCHUNK * BLK          # 8192 free cols per channel (interleaved layout)
YW = 2 * F                  # 8192 output cols per channel (re/im interleaved)
EPS = 1e-5

NQ = 4                      # load quarters per channel
QCH = NCHUNK // NQ          # chunks per quarter = 16
QW = QCH * BLK              # 2048 cols per quarter tile
QOUT = QCH * 2 * CHUNK      # 2048 output cols per quarter
HCH = QCH // 2              # chunks per whiten block (half quarter) = 8
HOUT = HCH * 2 * CHUNK      # 1024 output cols per whiten block

_CACHE = {}
_TRACE = False   # test.py sets this to capture NTFF profile / HW exec time
LAST = {}        # kernel() stores exec_time_ns etc. here

# tuning knobs
XY_BUFS = 4      # quarter tiles in flight per quarter-slot (ring per tag)
Y_BUFS = 12      # whiten output tiles in flight


def _build_nc():
    import concourse.bacc as bacc
    import concourse.mybir as mybir
    from concourse.tile import TileContext

    f32 = mybir.dt.float32
    f32r = mybir.dt.float32r
    Alu = mybir.AluOpType
    Act = mybir.ActivationFunctionType
    Axis = mybir.AxisListType

    nc = bacc.Bacc("TRN2", target_bir_lowering=False)
    # xy carries float32 bits but is declared float32r end-to-end so the BIR
    # verifier accepts it as a (fast-path) FP32r matmul operand.
    xy_d = nc.declare_dram_parameter("xy", [CLOC, P, XYW], f32r, isOutput=False)
    gb_d = nc.declare_dram_parameter("gb", [P, 48], f32, isOutput=False)
    y_d = nc.declare_dram_parameter("y", [CLOC, P, YW], f32, isOutput=True)

    V = nc.vector
    A = nc.scalar
    GS = nc.gpsimd

    rN = 1.0 / N
    rN1 = 1.0 / (N - 1)

    with TileContext(nc) as tc:
        with (
            tc.tile_pool(name="singles", bufs=1) as singles,
            tc.tile_pool(name="xyp", bufs=XY_BUFS) as xyp,
            tc.tile_pool(name="yp", bufs=Y_BUFS) as yp,
            tc.tile_pool(name="spp", bufs=2) as spp,
            tc.tile_pool(name="smallp", bufs=2) as smallp,
            tc.tile_pool(name="gramp", bufs=2, space="PSUM") as gramp,
            tc.tile_pool(name="g2p", bufs=2, space="PSUM") as g2p,
            tc.tile_pool(name="foldp", bufs=2, space="PSUM") as foldp,
        ):
            # gb rides the ACT queue so the SP queue's first entry is
            # channel 0's first big load.
            gb = singles.tile([P, 48], f32)
            A.dma_start(out=gb[:], in_=gb_d[:])

            # Partition-fold masks: mask_t sums partitions 0:64 into every
            # output partition, mask_b sums partitions 64:128.
            mask_t = singles.tile([P, P], f32)
            GS.memset(mask_t[0:CHUNK, :], 1.0)
            GS.memset(mask_t[CHUNK:P, :], 0.0)
            mask_b = singles.tile([P, P], f32)
            GS.memset(mask_b[0:CHUNK, :], 0.0)
            GS.memset(mask_b[CHUNK:P, :], 1.0)

            # Identity mask (ident[p, j] = 1 iff j == p mod 64), built
            # on-chip on Pool: two full-height affine diagonal selects
            # (predicates j - p == 0 and j - p + 64 == 0) summed. Full-height
            # APs only, so the result does not depend on whether the affine
            # channel index is absolute or AP-relative (a known sim/HW
            # divergence risk). Saves the constant-table DMA entirely.
            ones64 = singles.tile([P, CHUNK], f32)
            GS.memset(ones64[:], 1.0)
            diag_t = singles.tile([P, CHUNK], f32)
            GS.affine_select(diag_t[:], ones64[:],
                             pattern=[[1, CHUNK]], compare_op=Alu.is_equal,
                             fill=0.0, base=0, channel_multiplier=-1)
            ident = singles.tile([P, CHUNK], f32)
            GS.affine_select(ident[:], ones64[:],
                             pattern=[[1, CHUNK]], compare_op=Alu.is_equal,
                             fill=0.0, base=CHUNK, channel_multiplier=-1)
            GS.tensor_add(ident[:], ident[:], diag_t[:])
            # [ones | zeros] moving operand for the per-chunk col-sum
            # matmuls (the zero column keeps the moving width even).
            ones2 = singles.tile([P, 2], f32)
            GS.memset(ones2[:, 0:1], 1.0)
            GS.memset(ones2[:, 1:2], 0.0)

            # Warm the ACT function table off the critical path: sqrt first
            # so the chosen set (sqrt_and_others) also covers Identity/Copy.
            warm = singles.tile([P, 1], f32)
            A.sqrt(warm[:], gb[:, 0:1])
            A.activation(warm[:], gb[:, 0:1], Act.Identity,
                         bias=gb[:, 0:1], scale=1.0)

            state = {}

            def emit_loads(c):
                xq = []
                for q in range(NQ):
                    xt = xyp.tile([P, QW], f32r, tag=f"xy{q}")
                    nc.sync.dma_start(
                        out=xt[:], in_=xy_d[c][:, q * QW:(q + 1) * QW])
                    xq.append(xt)
                return xq

            def emit_gram(c, xq):
                g = gramp.tile([P, 2 * BLK], f32, tag="g")
                g2 = g2p.tile([P, 2], f32, tag="g2")
                state[c]["g2"] = g2
                for q in range(NQ):
                    for j in range(QCH):
                        w = 2 * BLK if j < QCH - 1 else BLK
                        first = q == 0 and j == 0
                        last = q == NQ - 1 and j == QCH - 1
                        lhsT = xq[q][:, j * BLK: j * BLK + 2 * CHUNK]
                        nc.tensor.matmul(
                            g[:, 0:w], lhsT=lhsT,
                            rhs=xq[q][:, j * BLK: j * BLK + w],
                            start=first, stop=last,
                        )
                        # col sums: same stationary, [ones|0] moving ->
                        # g2 accumulates per-column sums.
                        nc.tensor.matmul(
                            g2[:, :], lhsT=lhsT,
                            rhs=ones2[:].bitcast(f32r),
                            start=first, stop=last,
                        )
                    if q == 0 and c >= 2:
                        # Channel c-2's partition folds slot in between this
                        # channel's gram quarters: their input (extract(c-2))
                        # ran a full channel-period ago, so the PE never
                        # blocks on them. A fold placed any earlier stalls
                        # the PE queue on the extract it depends on, and the
                        # whole gram stream behind it.
                        emit_folds(c - 2)
                return g

            def emit_extract(c, g):
                # ---- extract: sp col layout:
                #   col 0 = [sum x^2 partials ; sum y^2 partials]
                #   col 1 = [col-sums of x    ; col-sums of y    ]
                #   col 2 = [sum x*y partials ; (zeroed)         ]
                #   col 3 = zero pad (keeps fold moving width even)
                sp = spp.tile([P, 4], f32, tag="sp")
                GS.memset(sp[CHUNK:P, 2:3], 0.0)
                GS.memset(sp[:, 3:4], 0.0)
                junk = smallp.tile([P, CHUNK], f32, tag="junk")
                V.tensor_mul(junk[0:CHUNK, :], g[0:CHUNK, 0:CHUNK],
                             ident[0:CHUNK, :])
                V.tensor_reduce(out=sp[0:CHUNK, 0:1], in_=junk[0:CHUNK, :],
                                axis=Axis.X, op=Alu.add)
                V.tensor_mul(junk[CHUNK:P, :], g[CHUNK:P, CHUNK:2 * CHUNK],
                             ident[CHUNK:P, :])
                V.tensor_reduce(out=sp[CHUNK:P, 0:1], in_=junk[CHUNK:P, :],
                                axis=Axis.X, op=Alu.add)
                V.tensor_mul(junk[0:CHUNK, :], g[0:CHUNK, CHUNK:2 * CHUNK],
                             ident[0:CHUNK, :])
                V.tensor_reduce(out=sp[0:CHUNK, 2:3], in_=junk[0:CHUNK, :],
                                axis=Axis.X, op=Alu.add)
                g2 = state[c]["g2"]
                A.copy(sp[0:CHUNK, 1:2], g2[0:CHUNK, 0:1])
                A.copy(sp[CHUNK:P, 1:2], g2[CHUNK:P, 0:1])
                return sp

            def emit_folds(c):
                sp = state[c]["sp"]
                sfold = foldp.tile([P, 8], f32, tag="f")
                nc.tensor.matmul(sfold[:, 0:4], lhsT=mask_t[:], rhs=sp[:, :],
                                 start=True, stop=True)
                nc.tensor.matmul(sfold[:, 4:8], lhsT=mask_b[:], rhs=sp[:, :],
                                 start=True, stop=True)
                state[c]["sfold"] = sfold

            def emit_asm(c):
                # The last channels' assembly runs during the load->store
                # transition where its serial latency is exposed; DVE's
                # dispatch (~70 ns/op) beats Pool's Q7 launch (95 ns/op +
                # overhead) there. Earlier channels stay on Pool to keep
                # DVE under its steady-state budget.
                E = V if c >= CLOC - 2 else GS
                sfold = state[c]["sfold"]
                # ACT stages the fold PSUM into SBUF so the Pool-engine
                # assembly never touches PSUM.
                ssb = smallp.tile([P, 8], f32, tag="ssb")
                A.copy(ssb[:], sfold[:])
                SXX, SR, SXY = ssb[:, 0:1], ssb[:, 1:2], ssb[:, 2:3]
                SYY, SI = ssb[:, 4:5], ssb[:, 5:6]

                # ---- 2x2 assembly on the (otherwise idle) Pool engine,
                # replicated across partitions; sqrts on ACT ----
                tmp = smallp.tile([P, 16], f32, tag="tmp")

                def ts(i, tmp=tmp):
                    return tmp[:, i:i + 1]

                MR, MI, u = ts(0), ts(1), ts(2)
                a, bb, cc = ts(3), ts(4), ts(5)
                E.tensor_scalar_mul(MR, SR, rN)
                E.tensor_scalar_mul(MI, SI, rN)
                E.tensor_mul(u, SR, MR)
                E.tensor_sub(a, SXX, u)
                E.tensor_scalar(out=a, in0=a, scalar1=rN1, scalar2=EPS,
                                 op0=Alu.mult, op1=Alu.add)
                E.tensor_mul(u, SR, MI)
                E.tensor_sub(bb, SXY, u)
                E.tensor_scalar_mul(bb, bb, rN1)
                E.tensor_mul(u, SI, MI)
                E.tensor_sub(cc, SYY, u)
                E.tensor_scalar(out=cc, in0=cc, scalar1=rN1, scalar2=EPS,
                                 op0=Alu.mult, op1=Alu.add)
                # (M)^{-1/2} for M=[[a,b],[b,c]]: s=sqrt(ac-b^2);
                # t=sqrt(a+c+2s); W=[[c+s,-b],[-b,a+s]]/(s*t)
                det, tr, tr2, st = ts(6), ts(7), ts(8), ts(9)
                E.tensor_mul(det, a, cc)
                E.tensor_mul(u, bb, bb)
                E.tensor_sub(det, det, u)
                E.tensor_add(tr, a, cc)
                sA = smallp.tile([P, 1], f32, tag="sA")
                tA = smallp.tile([P, 1], f32, tag="tA")
                A.sqrt(sA[:], det)
                E.tensor_add(u, sA[:], sA[:])
                E.tensor_add(tr2, u, tr)
                A.sqrt(tA[:], tr2)
                E.tensor_mul(st, sA[:], tA[:])
                # 1/(s*t) on DVE (Pool has no divide ALU); the one-op hop
                # sits between extract and whiten stage-2 in the DVE stream.
                inv = smallp.tile([P, 1], f32, tag="inv")
                V.reciprocal(inv[:], st)
                w00, w01, w11, q1, q2 = ts(10), ts(11), ts(12), ts(13), ts(14)
                E.tensor_add(w00, cc, sA[:])
                E.tensor_mul(w00, w00, inv[:])
                E.tensor_mul(w01, bb, inv[:])
                E.tensor_scalar_mul(w01, w01, -1.0)
                E.tensor_add(w11, a, sA[:])
                E.tensor_mul(w11, w11, inv[:])
                # G = gamma @ W ; B' = beta - G @ mean
                g00 = gb[:, 0 * 8 + c: 0 * 8 + c + 1]
                g01 = gb[:, 1 * 8 + c: 1 * 8 + c + 1]
                g10 = gb[:, 2 * 8 + c: 2 * 8 + c + 1]
                g11 = gb[:, 3 * 8 + c: 3 * 8 + c + 1]
                br_ = gb[:, 4 * 8 + c: 4 * 8 + c + 1]
                bi_ = gb[:, 5 * 8 + c: 5 * 8 + c + 1]
                cb = smallp.tile([P, 6], f32, tag="cb")
                G00, G01, BR = cb[:, 0:1], cb[:, 1:2], cb[:, 2:3]
                G10, G11, BI = cb[:, 3:4], cb[:, 4:5], cb[:, 5:6]
                E.tensor_mul(q1, g00, w00)
                E.tensor_mul(q2, g01, w01)
                E.tensor_add(G00, q1, q2)
                E.tensor_mul(q1, g00, w01)
                E.tensor_mul(q2, g01, w11)
                E.tensor_add(G01, q1, q2)
                E.tensor_mul(q1, g10, w00)
                E.tensor_mul(q2, g11, w01)
                E.tensor_add(G10, q1, q2)
                E.tensor_mul(q1, g10, w01)
                E.tensor_mul(q2, g11, w11)
                E.tensor_add(G11, q1, q2)
                E.tensor_mul(q1, G00, MR)
                E.tensor_mul(q2, G01, MI)
                E.tensor_add(q1, q1, q2)
                E.tensor_sub(BR, br_, q1)
                E.tensor_mul(q1, G10, MR)
                E.tensor_mul(q2, G11, MI)
                E.tensor_add(q1, q1, q2)
                E.tensor_sub(BI, bi_, q1)
                # Per-engine staged copies: whiten stage-1 (ACT) and stage-2
                # (DVE) read scale/bias scalars from tiles produced on their
                # own engine, so program order guarantees readiness.
                cbA = smallp.tile([P, 6], f32, tag="cbA")
                A.copy(cbA[:], cb[:])
                cbD = smallp.tile([P, 6], f32, tag="cbD")
                V.tensor_copy(cbD[:], cb[:])
                state[c]["cb"] = cb
                state[c]["cbA"] = cbA
                state[c]["cbD"] = cbD

            def emit_whiten(c):
                # ---- whiten + affine + store, per half-quarter block ----
                xq = state[c]["xq"]
                cbA, cbD = state[c]["cbA"], state[c]["cbD"]
                for h in range(2 * NQ):
                    q, hh = h // 2, h % 2
                    x3 = xq[q][:].bitcast(f32).rearrange(
                        "p (j k) -> p j k", k=BLK)
                    xr = x3[:, hh * HCH:(hh + 1) * HCH, 0:CHUNK]
                    xi = x3[:, hh * HCH:(hh + 1) * HCH, CHUNK:2 * CHUNK]
                    yt = yp.tile([P, HCH, 2 * CHUNK], f32, tag="y")
                    ye = yt[:, :, 0:2 * CHUNK:2]
                    yo = yt[:, :, 1:2 * CHUNK:2]
                    A.activation(ye, xr, Act.Identity,
                                 bias=cbA[:, 2:3], scale=cbA[:, 0:1])
                    A.activation(yo, xr, Act.Identity,
                                 bias=cbA[:, 5:6], scale=cbA[:, 3:4])
                    V.scalar_tensor_tensor(out=ye, in0=xi,
                                           scalar=cbD[:, 1:2],
                                           in1=ye, op0=Alu.mult, op1=Alu.add)
                    V.scalar_tensor_tensor(out=yo, in0=xi,
                                           scalar=cbD[:, 4:5],
                                           in1=yo, op0=Alu.mult, op1=Alu.add)
                    # Late channels run after the last load has issued, so
                    # the idle SP queue takes every other store; Pool's
                    # SWDGE generation alone would pace the drain.
                    eng = nc.sync if (c >= CLOC - 3 and h % 2 == 1) else GS
                    eng.dma_start(
                        out=y_d[c][:, h * HOUT:(h + 1) * HOUT],
                        in_=yt[:].rearrange("p a b -> p (a b)"))

            # Lag-2 software pipeline. Per iteration i the engine streams see:
            #   SP:   loads(i)
            #   PE:   gram(i) [folds(i-2) slotted in after quarter 0]
            #   DVE:  extract(i-1), asm(i-2), whiten-stage2(i-2)
            #   ACT:  colsum-copies(i-1), sqrts/cbA(i-2), whiten-stage1(i-2)
            #   Pool: sp-memsets(i-1), stores(i-2)
            # Every queue head's semaphore wait targets work from >= one
            # channel-period earlier, so no engine ever idles with ready
            # work parked behind a blocked instruction.
            for i in range(CLOC + 2):
                if i < CLOC:
                    state[i] = {}
                    state[i]["xq"] = emit_loads(i)
                    state[i]["g"] = emit_gram(i, state[i]["xq"])
                else:
                    emit_folds(i - 2)
                if 1 <= i <= CLOC:
                    state[i - 1]["sp"] = emit_extract(i - 1,
                                                      state[i - 1]["g"])
                if i >= 2:
                    emit_asm(i - 2)
                    emit_whiten(i - 2)

    nc.finalize()
    return nc


def _get_nc():
    if "nc" not in _CACHE:
        _CACHE["nc"] = _build_nc()
    return _CACHE["nc"]


def _prep_core(x_real, x_imag, gamma, beta, k):
    c0 = k * CLOC
    xr = np.ascontiguousarray(
        x_real[:, c0:c0 + CLOC].transpose(1, 0, 2, 3)
    ).reshape(CLOC, P, NCHUNK, CHUNK)
    xi = np.ascontiguousarray(
        x_imag[:, c0:c0 + CLOC].transpose(1, 0, 2, 3)
    ).reshape(CLOC, P, NCHUNK, CHUNK)
    xy = np.empty((CLOC, P, NCHUNK, BLK), np.float32)
    xy[..., 0:CHUNK] = xr
    xy[..., CHUNK:2 * CHUNK] = xi
    g = gamma[c0:c0 + CLOC]
    b = beta[c0:c0 + CLOC]
    gb = np.concatenate([g[:, 0, 0], g[:, 0, 1], g[:, 1, 0], g[:, 1, 1],
                         b[:, 0], b[:, 1]]).astype(np.float32).reshape(1, 48)
    gb = np.broadcast_to(gb, (P, 48)).copy()
    return {"xy": xy.reshape(CLOC, P, XYW), "gb": gb}


def kernel(x_real, x_imag, gamma, beta):
    from concourse.bass_utils import run_bass_kernel_spmd

    x_real = np.asarray(x_real, dtype=np.float32)
    x_imag = np.asarray(x_imag, dtype=np.float32)
    gamma = np.asarray(gamma, dtype=np.float32)
    beta = np.asarray(beta, dtype=np.float32)

    in_maps = [_prep_core(x_real, x_imag, gamma, beta, k)
               for k in range(NCORES)]

    nc = _get_nc()
    res = None
    if _TRACE:
        try:
            res = run_bass_kernel_spmd(nc, in_maps, list(range(NCORES)),
                                       trace=True)
        except Exception as e:  # trace infra unavailable -> plain run
            LAST["trace_error"] = repr(e)
            res = None
    if res is None:
        res = run_bass_kernel_spmd(nc, in_maps, list(range(NCORES)))
    LAST["exec_time_ns"] = res.exec_time_ns
    LAST["mean_exec_time_ns"] = res.mean_exec_time_ns
    LAST["profile_json"] = res.profile_json

    out = np.empty((B, C, H, W, 2), np.float32)
    for k in range(NCORES):
        c0 = k * CLOC
        y = res.results[k]["y"].reshape(CLOC, N, 2).reshape(CLOC, B, H, W, 2)
        out[:, c0:c0 + CLOC] = y.transpose(1, 0, 2, 3, 4)
    return out


# revision 8
# speedup vs baseline: 1.0041x; 1.0007x over previous
"""ComplexBatchNorm2d (Trabelsi-style complex whitening BN) on 8 trn2 NeuronCores.

Sharding: over channels C (8 channels per core). Each channel's batch statistics
are computed entirely on one core, so no collectives are needed.

v2 schedule (vs v1): the DMA engines are the roofline (67.6 MB/core at
360 GB/s = 188.7 us), so every other engine is organized to never make a
DMA wait:
  - channel data is loaded in QUARTER tiles (4 DMAs per channel, SP queue)
    so the gram matmuls start ~9 us into each load instead of after it;
  - y stores are issued from the Pool (SWDGE) queue -- and for the last
    channels alternately from the then-idle SP queue -- so a store waiting
    on compute can never stall a load;
  - stats extraction is 3 masked mul+reduce pairs (DVE) + 2 ACT copies;
    two masked fold matmuls replace the per-channel ones-matmul+memset;
  - the 2x2 assembly runs on the otherwise-idle Pool engine (sqrts on
    ACT, one reciprocal on DVE), lag-2 software-pipelined so no engine
    queue ever blocks on a not-ready wait;
  - no DRAM bounce for the coefficients: DVE consumers read the DVE-written
    cb tile (same-engine program order), ACT consumers read an ACT-staged
    copy;
  - whiten is split: ACT computes y = xr*G + B' via Identity activation
    (AP scale/bias), DVE adds xi*G' in place, halving DVE work.

Host side: slices/permutes inputs per core, builds the interleaved chunk
layout, gathers per-core outputs and permutes back to (B, C, H, W, 2).
"""

import numpy as np

# Problem geometry (hardcoded per contract).
B, C, H, W = 32, 64, 128, 128
NCORES = 8
CLOC = C // NCORES          # channels per core = 8
P = 128                     # SBUF partitions
N = B * H * W               # samples per channel = 524288
F = N // P                  # free columns per channel = 4096
CHUNK = 64                  # data columns per gram chunk
NCHUNK = F // CHUNK         # 64 chunks per channel
# [X(64) | Y(64)] per chunk -- pure data, no ones/pad columns. Column sums
# (for the means) come from a second accumulating matmul per chunk against
# a constant [P, 2] ones tile, so the loads carry zero layout overhead.
BLK = 2 * CHUNK             # 128 cols per chunk
XYW = NCHUNK * BLK          # 8192 free cols per channel (interleaved layout)
YW = 2 * F                  # 8192 output cols per channel (re/im interleaved)
EPS = 1e-5

NQ = 4                      # load quarters per channel
QCH = NCHUNK // NQ          # chunks per quarter = 16
QW = QCH * BLK              # 2048 cols per quarter tile
QOUT = QCH * 2 * CHUNK      # 2048 output cols per quarter
HCH = QCH // 2              # chunks per whiten block (half quarter) = 8
HOUT = HCH * 2 * CHUNK      # 1024 output cols per whiten block

_CACHE = {}
_TRACE = False   # test.py sets this to capture NTFF profile / HW exec time
LAST = {}        # kernel() stores exec_time_ns etc. here

# tuning knobs
XY_BUFS = 4      # quarter tiles in flight per quarter-slot (ring per tag)
Y_BUFS = 12      # whiten output tiles in flight


def _build_nc():
    import concourse.bacc as bacc
    import concourse.mybir as mybir
    from concourse.tile import TileContext

    f32 = mybir.dt.float32
    f32r = mybir.dt.float32r
    Alu = mybir.AluOpType
    Act = mybir.ActivationFunctionType
    Axis = mybir.AxisListType

    nc = bacc.Bacc("TRN2", target_bir_lowering=False)
    # xy carries float32 bits but is declared float32r end-to-end so the BIR
    # verifier accepts it as a (fast-path) FP32r matmul operand.
    xy_d = nc.declare_dram_parameter("xy", [CLOC, P, XYW], f32r, isOutput=False)
    gb_d = nc.declare_dram_parameter("gb", [1, 48], f32, isOutput=False)
    y_d = nc.declare_dram_parameter("y", [CLOC, P, YW], f32, isOutput=True)

    V = nc.vector
    A = nc.scalar
    GS = nc.gpsimd

    rN = 1.0 / N
    rN1 = 1.0 / (N - 1)

    with TileContext(nc) as tc:
        with (
            tc.tile_pool(name="singles", bufs=1) as singles,
            tc.tile_pool(name="xyp", bufs=XY_BUFS) as xyp,
            tc.tile_pool(name="yp", bufs=Y_BUFS) as yp,
            tc.tile_pool(name="spp", bufs=2) as spp,
            tc.tile_pool(name="smallp", bufs=2) as smallp,
            tc.tile_pool(name="gramp", bufs=2, space="PSUM") as gramp,
            tc.tile_pool(name="g2p", bufs=2, space="PSUM") as g2p,
            tc.tile_pool(name="foldp", bufs=2, space="PSUM") as foldp,
        ):
            # gb arrives as a single partition row (192 B instead of a
            # host-broadcast 24.5 KB) and is replicated across partitions
            # on-chip via the mask_t fold matmul (partition 0 is in its
            # summed range). Rides the ACT queue so the SP queue's first
            # entry is channel 0's first big load.
            gb0 = singles.tile([P, 48], f32)
            GS.memset(gb0[:], 0.0)
            A.dma_start(out=gb0[0:1, :], in_=gb_d[:])

            # Partition-fold masks: mask_t sums partitions 0:64 into every
            # output partition, mask_b sums partitions 64:128.
            mask_t = singles.tile([P, P], f32)
            GS.memset(mask_t[0:CHUNK, :], 1.0)
            GS.memset(mask_t[CHUNK:P, :], 0.0)
            mask_b = singles.tile([P, P], f32)
            GS.memset(mask_b[0:CHUNK, :], 0.0)
            GS.memset(mask_b[CHUNK:P, :], 1.0)

            # Identity mask (ident[p, j] = 1 iff j == p mod 64), built
            # on-chip on Pool: two full-height affine diagonal selects
            # (predicates j - p == 0 and j - p + 64 == 0) summed. Full-height
            # APs only, so the result does not depend on whether the affine
            # channel index is absolute or AP-relative (a known sim/HW
            # divergence risk). Saves the constant-table DMA entirely.
            ones64 = singles.tile([P, CHUNK], f32)
            GS.memset(ones64[:], 1.0)
            diag_t = singles.tile([P, CHUNK], f32)
            GS.affine_select(diag_t[:], ones64[:],
                             pattern=[[1, CHUNK]], compare_op=Alu.is_equal,
                             fill=0.0, base=0, channel_multiplier=-1)
            ident = singles.tile([P, CHUNK], f32)
            GS.affine_select(ident[:], ones64[:],
                             pattern=[[1, CHUNK]], compare_op=Alu.is_equal,
                             fill=0.0, base=CHUNK, channel_multiplier=-1)
            GS.tensor_add(ident[:], ident[:], diag_t[:])
            # [ones | zeros] moving operand for the per-chunk col-sum
            # matmuls (the zero column keeps the moving width even).
            ones2 = singles.tile([P, 2], f32)
            GS.memset(ones2[:, 0:1], 1.0)
            GS.memset(ones2[:, 1:2], 0.0)

            gbp = foldp.tile([P, 48], f32, tag="gbb")
            nc.tensor.matmul(gbp[:, :], lhsT=mask_t[:], rhs=gb0[:, :],
                             start=True, stop=True)
            gb = singles.tile([P, 48], f32)
            A.copy(gb[:], gbp[:, :])

            # Warm the ACT function table off the critical path: sqrt first
            # so the chosen set (sqrt_and_others) also covers Identity/Copy.
            warm = singles.tile([P, 1], f32)
            A.sqrt(warm[:], gb[:, 0:1])
            A.activation(warm[:], gb[:, 0:1], Act.Identity,
                         bias=gb[:, 0:1], scale=1.0)

            state = {}

            def emit_loads(c):
                xq = []
                for q in range(NQ):
                    xt = xyp.tile([P, QW], f32r, tag=f"xy{q}")
                    nc.sync.dma_start(
                        out=xt[:], in_=xy_d[c][:, q * QW:(q + 1) * QW])
                    xq.append(xt)
                return xq

            def emit_gram(c, xq):
                g = gramp.tile([P, 2 * BLK], f32, tag="g")
                g2 = g2p.tile([P, 2], f32, tag="g2")
                state[c]["g2"] = g2
                for q in range(NQ):
                    for j in range(QCH):
                        w = 2 * BLK if j < QCH - 1 else BLK
                        first = q == 0 and j == 0
                        last = q == NQ - 1 and j == QCH - 1
                        lhsT = xq[q][:, j * BLK: j * BLK + 2 * CHUNK]
                        nc.tensor.matmul(
                            g[:, 0:w], lhsT=lhsT,
                            rhs=xq[q][:, j * BLK: j * BLK + w],
                            start=first, stop=last,
                        )
                        # col sums: same stationary, [ones|0] moving ->
                        # g2 accumulates per-column sums.
                        nc.tensor.matmul(
                            g2[:, :], lhsT=lhsT,
                            rhs=ones2[:].bitcast(f32r),
                            start=first, stop=last,
                        )
                    if q == 0 and c >= 2:
                        # Channel c-2's partition folds slot in between this
                        # channel's gram quarters: their input (extract(c-2))
                        # ran a full channel-period ago, so the PE never
                        # blocks on them. A fold placed any earlier stalls
                        # the PE queue on the extract it depends on, and the
                        # whole gram stream behind it.
                        emit_folds(c - 2)
                return g

            def emit_extract(c, g):
                # ---- extract: sp col layout:
                #   col 0 = [sum x^2 partials ; sum y^2 partials]
                #   col 1 = [col-sums of x    ; col-sums of y    ]
                #   col 2 = [sum x*y partials ; (zeroed)         ]
                #   col 3 = zero pad (keeps fold moving width even)
                sp = spp.tile([P, 4], f32, tag="sp")
                GS.memset(sp[CHUNK:P, 2:3], 0.0)
                GS.memset(sp[:, 3:4], 0.0)
                junk = smallp.tile([P, CHUNK], f32, tag="junk")
                V.tensor_mul(junk[0:CHUNK, :], g[0:CHUNK, 0:CHUNK],
                             ident[0:CHUNK, :])
                V.tensor_reduce(out=sp[0:CHUNK, 0:1], in_=junk[0:CHUNK, :],
                                axis=Axis.X, op=Alu.add)
                V.tensor_mul(junk[CHUNK:P, :], g[CHUNK:P, CHUNK:2 * CHUNK],
                             ident[CHUNK:P, :])
                V.tensor_reduce(out=sp[CHUNK:P, 0:1], in_=junk[CHUNK:P, :],
                                axis=Axis.X, op=Alu.add)
                V.tensor_mul(junk[0:CHUNK, :], g[0:CHUNK, CHUNK:2 * CHUNK],
                             ident[0:CHUNK, :])
                V.tensor_reduce(out=sp[0:CHUNK, 2:3], in_=junk[0:CHUNK, :],
                                axis=Axis.X, op=Alu.add)
                g2 = state[c]["g2"]
                A.copy(sp[0:CHUNK, 1:2], g2[0:CHUNK, 0:1])
                A.copy(sp[CHUNK:P, 1:2], g2[CHUNK:P, 0:1])
                return sp

            def emit_folds(c):
                sp = state[c]["sp"]
                sfold = foldp.tile([P, 8], f32, tag="f")
                nc.tensor.matmul(sfold[:, 0:4], lhsT=mask_t[:], rhs=sp[:, :],
                                 start=True, stop=True)
                nc.tensor.matmul(sfold[:, 4:8], lhsT=mask_b[:], rhs=sp[:, :],
                                 start=True, stop=True)
                state[c]["sfold"] = sfold

            def emit_asm(c):
                # The last channels' assembly runs during the load->store
                # transition where its serial latency is exposed; DVE's
                # dispatch (~70 ns/op) beats Pool's Q7 launch (95 ns/op +
                # overhead) there. Earlier channels stay on Pool to keep
                # DVE under its steady-state budget.
                E = V if c >= CLOC - 2 else GS
                sfold = state[c]["sfold"]
                # ACT stages the fold PSUM into SBUF so the Pool-engine
                # assembly never touches PSUM.
                ssb = smallp.tile([P, 8], f32, tag="ssb")
                A.copy(ssb[:], sfold[:])
                SXX, SR, SXY = ssb[:, 0:1], ssb[:, 1:2], ssb[:, 2:3]
                SYY, SI = ssb[:, 4:5], ssb[:, 5:6]

                # ---- 2x2 assembly on the (otherwise idle) Pool engine,
                # replicated across partitions; sqrts on ACT ----
                tmp = smallp.tile([P, 16], f32, tag="tmp")

                def ts(i, tmp=tmp):
                    return tmp[:, i:i + 1]

                MR, MI, u = ts(0), ts(1), ts(2)
                a, bb, cc = ts(3), ts(4), ts(5)
                E.tensor_scalar_mul(MR, SR, rN)
                E.tensor_scalar_mul(MI, SI, rN)
                E.tensor_mul(u, SR, MR)
                E.tensor_sub(a, SXX, u)
                E.tensor_scalar(out=a, in0=a, scalar1=rN1, scalar2=EPS,
                                 op0=Alu.mult, op1=Alu.add)
                E.tensor_mul(u, SR, MI)
                E.tensor_sub(bb, SXY, u)
                E.tensor_scalar_mul(bb, bb, rN1)
                E.tensor_mul(u, SI, MI)
                E.tensor_sub(cc, SYY, u)
                E.tensor_scalar(out=cc, in0=cc, scalar1=rN1, scalar2=EPS,
                                 op0=Alu.mult, op1=Alu.add)
                # (M)^{-1/2} for M=[[a,b],[b,c]]: s=sqrt(ac-b^2);
                # t=sqrt(a+c+2s); W=[[c+s,-b],[-b,a+s]]/(s*t)
                det, tr, tr2, st = ts(6), ts(7), ts(8), ts(9)
                E.tensor_mul(det, a, cc)
                E.tensor_mul(u, bb, bb)
                E.tensor_sub(det, det, u)
                E.tensor_add(tr, a, cc)
                sA = smallp.tile([P, 1], f32, tag="sA")
                tA = smallp.tile([P, 1], f32, tag="tA")
                A.sqrt(sA[:], det)
                E.tensor_add(u, sA[:], sA[:])
                E.tensor_add(tr2, u, tr)
                A.sqrt(tA[:], tr2)
                E.tensor_mul(st, sA[:], tA[:])
                # 1/(s*t) on DVE (Pool has no divide ALU); the one-op hop
                # sits between extract and whiten stage-2 in the DVE stream.
                inv = smallp.tile([P, 1], f32, tag="inv")
                V.reciprocal(inv[:], st)
                w00, w01, w11, q1, q2 = ts(10), ts(11), ts(12), ts(13), ts(14)
                E.tensor_add(w00, cc, sA[:])
                E.tensor_mul(w00, w00, inv[:])
                E.tensor_mul(w01, bb, inv[:])
                E.tensor_scalar_mul(w01, w01, -1.0)
                E.tensor_add(w11, a, sA[:])
                E.tensor_mul(w11, w11, inv[:])
                # G = gamma @ W ; B' = beta - G @ mean
                g00 = gb[:, 0 * 8 + c: 0 * 8 + c + 1]
                g01 = gb[:, 1 * 8 + c: 1 * 8 + c + 1]
                g10 = gb[:, 2 * 8 + c: 2 * 8 + c + 1]
                g11 = gb[:, 3 * 8 + c: 3 * 8 + c + 1]
                br_ = gb[:, 4 * 8 + c: 4 * 8 + c + 1]
                bi_ = gb[:, 5 * 8 + c: 5 * 8 + c + 1]
                cb = smallp.tile([P, 6], f32, tag="cb")
                G00, G01, BR = cb[:, 0:1], cb[:, 1:2], cb[:, 2:3]
                G10, G11, BI = cb[:, 3:4], cb[:, 4:5], cb[:, 5:6]
                E.tensor_mul(q1, g00, w00)
                E.tensor_mul(q2, g01, w01)
                E.tensor_add(G00, q1, q2)
                E.tensor_mul(q1, g00, w01)
                E.tensor_mul(q2, g01, w11)
                E.tensor_add(G01, q1, q2)
                E.tensor_mul(q1, g10, w00)
                E.tensor_mul(q2, g11, w01)
                E.tensor_add(G10, q1, q2)
                E.tensor_mul(q1, g10, w01)
                E.tensor_mul(q2, g11, w11)
                E.tensor_add(G11, q1, q2)
                E.tensor_mul(q1, G00, MR)
                E.tensor_mul(q2, G01, MI)
                E.tensor_add(q1, q1, q2)
                E.tensor_sub(BR, br_, q1)
                E.tensor_mul(q1, G10, MR)
                E.tensor_mul(q2, G11, MI)
                E.tensor_add(q1, q1, q2)
                E.tensor_sub(BI, bi_, q1)
                # Per-engine staged copies: whiten stage-1 (ACT) and stage-2
                # (DVE) read scale/bias scalars from tiles produced on their
                # own engine, so program order guarantees readiness.
                cbA = smallp.tile([P, 6], f32, tag="cbA")
                A.copy(cbA[:], cb[:])
                cbD = smallp.tile([P, 6], f32, tag="cbD")
                V.tensor_copy(cbD[:], cb[:])
                state[c]["cb"] = cb
                state[c]["cbA"] = cbA
                state[c]["cbD"] = cbD

            def emit_whiten(c):
                # ---- whiten + affine + store, per half-quarter block ----
                xq = state[c]["xq"]
                cbA, cbD = state[c]["cbA"], state[c]["cbD"]
                for h in range(2 * NQ):
                    q, hh = h // 2, h % 2
                    x3 = xq[q][:].bitcast(f32).rearrange(
                        "p (j k) -> p j k", k=BLK)
                    xr = x3[:, hh * HCH:(hh + 1) * HCH, 0:CHUNK]
                    xi = x3[:, hh * HCH:(hh + 1) * HCH, CHUNK:2 * CHUNK]
                    yt = yp.tile([P, HCH, 2 * CHUNK], f32, tag="y")
                    ye = yt[:, :, 0:2 * CHUNK:2]
                    yo = yt[:, :, 1:2 * CHUNK:2]
                    A.activation(ye, xr, Act.Identity,
                                 bias=cbA[:, 2:3], scale=cbA[:, 0:1])
                    A.activation(yo, xr, Act.Identity,
                                 bias=cbA[:, 5:6], scale=cbA[:, 3:4])
                    V.scalar_tensor_tensor(out=ye, in0=xi,
                                           scalar=cbD[:, 1:2],
                                           in1=ye, op0=Alu.mult, op1=Alu.add)
                    V.scalar_tensor_tensor(out=yo, in0=xi,
                                           scalar=cbD[:, 4:5],
                                           in1=yo, op0=Alu.mult, op1=Alu.add)
                    # Late channels run after the last load has issued, so
                    # the idle SP queue takes every other store; Pool's
                    # SWDGE generation alone would pace the drain.
                    eng = nc.sync if (c >= CLOC - 3 and h % 2 == 1) else GS
                    eng.dma_start(
                        out=y_d[c][:, h * HOUT:(h + 1) * HOUT],
                        in_=yt[:].rearrange("p a b -> p (a b)"))

            # Lag-2 software pipeline. Per iteration i the engine streams see:
            #   SP:   loads(i)
            #   PE:   gram(i) [folds(i-2) slotted in after quarter 0]
            #   DVE:  extract(i-1), asm(i-2), whiten-stage2(i-2)
            #   ACT:  colsum-copies(i-1), sqrts/cbA(i-2), whiten-stage1(i-2)
            #   Pool: sp-memsets(i-1), stores(i-2)
            # Every queue head's semaphore wait targets work from >= one
            # channel-period earlier, so no engine ever idles with ready
            # work parked behind a blocked instruction.
            for i in range(CLOC + 2):
                if i < CLOC:
                    state[i] = {}
                    state[i]["xq"] = emit_loads(i)
                    state[i]["g"] = emit_gram(i, state[i]["xq"])
                else:
                    emit_folds(i - 2)
                if 1 <= i <= CLOC:
                    state[i - 1]["sp"] = emit_extract(i - 1,
                                                      state[i - 1]["g"])
                if i >= 2:
                    emit_asm(i - 2)
                    emit_whiten(i - 2)

    nc.finalize()
    return nc


def _get_nc():
    if "nc" not in _CACHE:
        _CACHE["nc"] = _build_nc()
    return _CACHE["nc"]


def _prep_core(x_real, x_imag, gamma, beta, k):
    c0 = k * CLOC
    xr = np.ascontiguousarray(
        x_real[:, c0:c0 + CLOC].transpose(1, 0, 2, 3)
    ).reshape(CLOC, P, NCHUNK, CHUNK)
    xi = np.ascontiguousarray(
        x_imag[:, c0:c0 + CLOC].transpose(1, 0, 2, 3)
    ).reshape(CLOC, P, NCHUNK, CHUNK)
    xy = np.empty((CLOC, P, NCHUNK, BLK), np.float32)
    xy[..., 0:CHUNK] = xr
    xy[..., CHUNK:2 * CHUNK] = xi
    g = gamma[c0:c0 + CLOC]
    b = beta[c0:c0 + CLOC]
    gb = np.concatenate([g[:, 0, 0], g[:, 0, 1], g[:, 1, 0], g[:, 1, 1],
                         b[:, 0], b[:, 1]]).astype(np.float32).reshape(1, 48)
    return {"xy": xy.reshape(CLOC, P, XYW), "gb": gb}


def kernel(x_real, x_imag, gamma, beta):
    from concourse.bass_utils import run_bass_kernel_spmd

    x_real = np.asarray(x_real, dtype=np.float32)
    x_imag = np.asarray(x_imag, dtype=np.float32)
    gamma = np.asarray(gamma, dtype=np.float32)
    beta = np.asarray(beta, dtype=np.float32)

    in_maps = [_prep_core(x_real, x_imag, gamma, beta, k)
               for k in range(NCORES)]

    nc = _get_nc()
    res = None
    if _TRACE:
        try:
            res = run_bass_kernel_spmd(nc, in_maps, list(range(NCORES)),
                                       trace=True)
        except Exception as e:  # trace infra unavailable -> plain run
            LAST["trace_error"] = repr(e)
            res = None
    if res is None:
        res = run_bass_kernel_spmd(nc, in_maps, list(range(NCORES)))
    LAST["exec_time_ns"] = res.exec_time_ns
    LAST["mean_exec_time_ns"] = res.mean_exec_time_ns
    LAST["profile_json"] = res.profile_json

    out = np.empty((B, C, H, W, 2), np.float32)
    for k in range(NCORES):
        c0 = k * CLOC
        y = res.results[k]["y"].reshape(CLOC, N, 2).reshape(CLOC, B, H, W, 2)
        out[:, c0:c0 + CLOC] = y.transpose(1, 0, 2, 3, 4)
    return out


# revision 9
# speedup vs baseline: 1.0045x; 1.0003x over previous
"""ComplexBatchNorm2d (Trabelsi-style complex whitening BN) on 8 trn2 NeuronCores.

Sharding: over channels C (8 channels per core). Each channel's batch statistics
are computed entirely on one core, so no collectives are needed.

v2 schedule (vs v1): the DMA engines are the roofline (67.6 MB/core at
360 GB/s = 188.7 us), so every other engine is organized to never make a
DMA wait:
  - channel data is loaded in QUARTER tiles (4 DMAs per channel, SP queue)
    so the gram matmuls start ~9 us into each load instead of after it;
  - y stores are issued from the Pool (SWDGE) queue -- and for the last
    channels alternately from the then-idle SP queue -- so a store waiting
    on compute can never stall a load;
  - stats extraction is 3 masked mul+reduce pairs (DVE) + 2 ACT copies;
    two masked fold matmuls replace the per-channel ones-matmul+memset;
  - the 2x2 assembly runs on the otherwise-idle Pool engine (sqrts on
    ACT, one reciprocal on DVE), lag-2 software-pipelined so no engine
    queue ever blocks on a not-ready wait;
  - no DRAM bounce for the coefficients: DVE consumers read the DVE-written
    cb tile (same-engine program order), ACT consumers read an ACT-staged
    copy;
  - whiten is split: ACT computes y = xr*G + B' via Identity activation
    (AP scale/bias), DVE adds xi*G' in place, halving DVE work.

Host side: slices/permutes inputs per core, builds the interleaved chunk
layout, gathers per-core outputs and permutes back to (B, C, H, W, 2).
"""

import numpy as np

# Problem geometry (hardcoded per contract).
B, C, H, W = 32, 64, 128, 128
NCORES = 8
CLOC = C // NCORES          # channels per core = 8
P = 128                     # SBUF partitions
N = B * H * W               # samples per channel = 524288
F = N // P                  # free columns per channel = 4096
CHUNK = 64                  # data columns per gram chunk
NCHUNK = F // CHUNK         # 64 chunks per channel
# [X(64) | Y(64)] per chunk -- pure data, no ones/pad columns. Column sums
# (for the means) come from a second accumulating matmul per chunk against
# a constant [P, 2] ones tile, so the loads carry zero layout overhead.
BLK = 2 * CHUNK             # 128 cols per chunk
XYW = NCHUNK * BLK          # 8192 free cols per channel (interleaved layout)
YW = 2 * F                  # 8192 output cols per channel (re/im interleaved)
EPS = 1e-5

NQ = 4                      # load quarters per channel
QCH = NCHUNK // NQ          # chunks per quarter = 16
QW = QCH * BLK              # 2048 cols per quarter tile
QOUT = QCH * 2 * CHUNK      # 2048 output cols per quarter
HCH = QCH // 2              # chunks per whiten block (half quarter) = 8
HOUT = HCH * 2 * CHUNK      # 1024 output cols per whiten block

_CACHE = {}
_TRACE = False   # test.py sets this to capture NTFF profile / HW exec time
LAST = {}        # kernel() stores exec_time_ns etc. here

# tuning knobs
XY_BUFS = 4      # quarter tiles in flight per quarter-slot (ring per tag)
Y_BUFS = 12      # whiten output tiles in flight


def _build_nc():
    import concourse.bacc as bacc
    import concourse.mybir as mybir
    from concourse.tile import TileContext

    f32 = mybir.dt.float32
    f32r = mybir.dt.float32r
    Alu = mybir.AluOpType
    Act = mybir.ActivationFunctionType
    Axis = mybir.AxisListType

    nc = bacc.Bacc("TRN2", target_bir_lowering=False)
    # xy carries float32 bits but is declared float32r end-to-end so the BIR
    # verifier accepts it as a (fast-path) FP32r matmul operand.
    xy_d = nc.declare_dram_parameter("xy", [CLOC, P, XYW], f32r, isOutput=False)
    gb_d = nc.declare_dram_parameter("gb", [1, 48], f32, isOutput=False)
    y_d = nc.declare_dram_parameter("y", [CLOC, P, YW], f32, isOutput=True)

    V = nc.vector
    A = nc.scalar
    GS = nc.gpsimd

    rN = 1.0 / N
    rN1 = 1.0 / (N - 1)

    with TileContext(nc) as tc:
        with (
            tc.tile_pool(name="singles", bufs=1) as singles,
            tc.tile_pool(name="xyp", bufs=XY_BUFS) as xyp,
            tc.tile_pool(name="yp", bufs=Y_BUFS) as yp,
            tc.tile_pool(name="spp", bufs=2) as spp,
            tc.tile_pool(name="smallp", bufs=2) as smallp,
            tc.tile_pool(name="gramp", bufs=2, space="PSUM") as gramp,
            tc.tile_pool(name="g2p", bufs=2, space="PSUM") as g2p,
            tc.tile_pool(name="foldp", bufs=2, space="PSUM") as foldp,
        ):
            # gb arrives as a single partition row (192 B instead of a
            # host-broadcast 24.5 KB) and is replicated across partitions
            # on-chip via the mask_t fold matmul (partition 0 is in its
            # summed range). Rides the ACT queue so the SP queue's first
            # entry is channel 0's first big load.
            gb0 = singles.tile([P, 48], f32)
            GS.memset(gb0[:], 0.0)
            A.dma_start(out=gb0[0:1, :], in_=gb_d[:])

            # Partition-fold masks: mask_t sums partitions 0:64 into every
            # output partition, mask_b sums partitions 64:128.
            mask_t = singles.tile([P, P], f32)
            GS.memset(mask_t[0:CHUNK, :], 1.0)
            GS.memset(mask_t[CHUNK:P, :], 0.0)
            mask_b = singles.tile([P, P], f32)
            GS.memset(mask_b[0:CHUNK, :], 0.0)
            GS.memset(mask_b[CHUNK:P, :], 1.0)

            # Identity mask (ident[p, j] = 1 iff j == p mod 64), built
            # on-chip on Pool from two f32 iotas (values <= 127 are exact in
            # f32): diff = j - p, then OR of the two diagonal predicates
            # (diff == 0, diff == -64). Full-height APs only, and no
            # affine_select: its constant fill emits a RegisterMove that the
            # scheduler hoists ahead of the preamble sem-init memsets,
            # delaying the init barrier by 61 ns.
            dif = singles.tile([P, CHUNK], f32)
            GS.iota(dif[:], pattern=[[1, CHUNK]], base=0,
                    channel_multiplier=-1,
                    allow_small_or_imprecise_dtypes=True)
            diag_t = singles.tile([P, CHUNK], f32)
            GS.tensor_scalar(out=diag_t[:], in0=dif[:], scalar1=0.0,
                             scalar2=None, op0=Alu.is_equal)
            ident = singles.tile([P, CHUNK], f32)
            GS.tensor_scalar(out=ident[:], in0=dif[:], scalar1=-float(CHUNK),
                             scalar2=None, op0=Alu.is_equal)
            GS.tensor_add(ident[:], ident[:], diag_t[:])
            # [ones | zeros] moving operand for the per-chunk col-sum
            # matmuls (the zero column keeps the moving width even).
            ones2 = singles.tile([P, 2], f32)
            GS.memset(ones2[:, 0:1], 1.0)
            GS.memset(ones2[:, 1:2], 0.0)

            gbp = foldp.tile([P, 48], f32, tag="gbb")
            nc.tensor.matmul(gbp[:, :], lhsT=mask_t[:], rhs=gb0[:, :],
                             start=True, stop=True)
            gb = singles.tile([P, 48], f32)
            A.copy(gb[:], gbp[:, :])

            # Warm the ACT function table off the critical path: sqrt first
            # so the chosen set (sqrt_and_others) also covers Identity/Copy.
            warm = singles.tile([P, 1], f32)
            A.sqrt(warm[:], gb[:, 0:1])
            A.activation(warm[:], gb[:, 0:1], Act.Identity,
                         bias=gb[:, 0:1], scale=1.0)

            state = {}

            def emit_loads(c):
                xq = []
                for q in range(NQ):
                    xt = xyp.tile([P, QW], f32r, tag=f"xy{q}")
                    nc.sync.dma_start(
                        out=xt[:], in_=xy_d[c][:, q * QW:(q + 1) * QW])
                    xq.append(xt)
                return xq

            def emit_gram(c, xq):
                g = gramp.tile([P, 2 * BLK], f32, tag="g")
                g2 = g2p.tile([P, 2], f32, tag="g2")
                state[c]["g2"] = g2
                for q in range(NQ):
                    for j in range(QCH):
                        w = 2 * BLK if j < QCH - 1 else BLK
                        first = q == 0 and j == 0
                        last = q == NQ - 1 and j == QCH - 1
                        lhsT = xq[q][:, j * BLK: j * BLK + 2 * CHUNK]
                        nc.tensor.matmul(
                            g[:, 0:w], lhsT=lhsT,
                            rhs=xq[q][:, j * BLK: j * BLK + w],
                            start=first, stop=last,
                        )
                        # col sums: same stationary, [ones|0] moving ->
                        # g2 accumulates per-column sums.
                        nc.tensor.matmul(
                            g2[:, :], lhsT=lhsT,
                            rhs=ones2[:].bitcast(f32r),
                            start=first, stop=last,
                        )
                    if q == 0 and c >= 2:
                        # Channel c-2's partition folds slot in between this
                        # channel's gram quarters: their input (extract(c-2))
                        # ran a full channel-period ago, so the PE never
                        # blocks on them. A fold placed any earlier stalls
                        # the PE queue on the extract it depends on, and the
                        # whole gram stream behind it.
                        emit_folds(c - 2)
                return g

            def emit_extract(c, g):
                # ---- extract: sp col layout:
                #   col 0 = [sum x^2 partials ; sum y^2 partials]
                #   col 1 = [col-sums of x    ; col-sums of y    ]
                #   col 2 = [sum x*y partials ; (zeroed)         ]
                #   col 3 = zero pad (keeps fold moving width even)
                sp = spp.tile([P, 4], f32, tag="sp")
                GS.memset(sp[CHUNK:P, 2:3], 0.0)
                GS.memset(sp[:, 3:4], 0.0)
                junk = smallp.tile([P, CHUNK], f32, tag="junk")
                V.tensor_mul(junk[0:CHUNK, :], g[0:CHUNK, 0:CHUNK],
                             ident[0:CHUNK, :])
                V.tensor_reduce(out=sp[0:CHUNK, 0:1], in_=junk[0:CHUNK, :],
                                axis=Axis.X, op=Alu.add)
                V.tensor_mul(junk[CHUNK:P, :], g[CHUNK:P, CHUNK:2 * CHUNK],
                             ident[CHUNK:P, :])
                V.tensor_reduce(out=sp[CHUNK:P, 0:1], in_=junk[CHUNK:P, :],
                                axis=Axis.X, op=Alu.add)
                V.tensor_mul(junk[0:CHUNK, :], g[0:CHUNK, CHUNK:2 * CHUNK],
                             ident[0:CHUNK, :])
                V.tensor_reduce(out=sp[0:CHUNK, 2:3], in_=junk[0:CHUNK, :],
                                axis=Axis.X, op=Alu.add)
                g2 = state[c]["g2"]
                A.copy(sp[0:CHUNK, 1:2], g2[0:CHUNK, 0:1])
                A.copy(sp[CHUNK:P, 1:2], g2[CHUNK:P, 0:1])
                return sp

            def emit_folds(c):
                sp = state[c]["sp"]
                sfold = foldp.tile([P, 8], f32, tag="f")
                nc.tensor.matmul(sfold[:, 0:4], lhsT=mask_t[:], rhs=sp[:, :],
                                 start=True, stop=True)
                nc.tensor.matmul(sfold[:, 4:8], lhsT=mask_b[:], rhs=sp[:, :],
                                 start=True, stop=True)
                state[c]["sfold"] = sfold

            def emit_asm(c):
                # The last channels' assembly runs during the load->store
                # transition where its serial latency is exposed; DVE's
                # dispatch (~70 ns/op) beats Pool's Q7 launch (95 ns/op +
                # overhead) there. Earlier channels stay on Pool to keep
                # DVE under its steady-state budget.
                E = V if c >= CLOC - 2 else GS
                sfold = state[c]["sfold"]
                # ACT stages the fold PSUM into SBUF so the Pool-engine
                # assembly never touches PSUM.
                ssb = smallp.tile([P, 8], f32, tag="ssb")
                A.copy(ssb[:], sfold[:])
                SXX, SR, SXY = ssb[:, 0:1], ssb[:, 1:2], ssb[:, 2:3]
                SYY, SI = ssb[:, 4:5], ssb[:, 5:6]

                # ---- 2x2 assembly on the (otherwise idle) Pool engine,
                # replicated across partitions; sqrts on ACT ----
                tmp = smallp.tile([P, 16], f32, tag="tmp")

                def ts(i, tmp=tmp):
                    return tmp[:, i:i + 1]

                MR, MI, u = ts(0), ts(1), ts(2)
                a, bb, cc = ts(3), ts(4), ts(5)
                E.tensor_scalar_mul(MR, SR, rN)
                E.tensor_scalar_mul(MI, SI, rN)
                E.tensor_mul(u, SR, MR)
                E.tensor_sub(a, SXX, u)
                E.tensor_scalar(out=a, in0=a, scalar1=rN1, scalar2=EPS,
                                 op0=Alu.mult, op1=Alu.add)
                E.tensor_mul(u, SR, MI)
                E.tensor_sub(bb, SXY, u)
                E.tensor_scalar_mul(bb, bb, rN1)
                E.tensor_mul(u, SI, MI)
                E.tensor_sub(cc, SYY, u)
                E.tensor_scalar(out=cc, in0=cc, scalar1=rN1, scalar2=EPS,
                                 op0=Alu.mult, op1=Alu.add)
                # (M)^{-1/2} for M=[[a,b],[b,c]]: s=sqrt(ac-b^2);
                # t=sqrt(a+c+2s); W=[[c+s,-b],[-b,a+s]]/(s*t)
                det, tr, tr2, st = ts(6), ts(7), ts(8), ts(9)
                E.tensor_mul(det, a, cc)
                E.tensor_mul(u, bb, bb)
                E.tensor_sub(det, det, u)
                E.tensor_add(tr, a, cc)
                sA = smallp.tile([P, 1], f32, tag="sA")
                tA = smallp.tile([P, 1], f32, tag="tA")
                A.sqrt(sA[:], det)
                E.tensor_add(u, sA[:], sA[:])
                E.tensor_add(tr2, u, tr)
                A.sqrt(tA[:], tr2)
                E.tensor_mul(st, sA[:], tA[:])
                # 1/(s*t) on DVE (Pool has no divide ALU); the one-op hop
                # sits between extract and whiten stage-2 in the DVE stream.
                inv = smallp.tile([P, 1], f32, tag="inv")
                V.reciprocal(inv[:], st)
                w00, w01, w11, q1, q2 = ts(10), ts(11), ts(12), ts(13), ts(14)
                E.tensor_add(w00, cc, sA[:])
                E.tensor_mul(w00, w00, inv[:])
                E.tensor_mul(w01, bb, inv[:])
                E.tensor_scalar_mul(w01, w01, -1.0)
                E.tensor_add(w11, a, sA[:])
                E.tensor_mul(w11, w11, inv[:])
                # G = gamma @ W ; B' = beta - G @ mean
                g00 = gb[:, 0 * 8 + c: 0 * 8 + c + 1]
                g01 = gb[:, 1 * 8 + c: 1 * 8 + c + 1]
                g10 = gb[:, 2 * 8 + c: 2 * 8 + c + 1]
                g11 = gb[:, 3 * 8 + c: 3 * 8 + c + 1]
                br_ = gb[:, 4 * 8 + c: 4 * 8 + c + 1]
                bi_ = gb[:, 5 * 8 + c: 5 * 8 + c + 1]
                cb = smallp.tile([P, 6], f32, tag="cb")
                G00, G01, BR = cb[:, 0:1], cb[:, 1:2], cb[:, 2:3]
                G10, G11, BI = cb[:, 3:4], cb[:, 4:5], cb[:, 5:6]
                E.tensor_mul(q1, g00, w00)
                E.tensor_mul(q2, g01, w01)
                E.tensor_add(G00, q1, q2)
                E.tensor_mul(q1, g00, w01)
                E.tensor_mul(q2, g01, w11)
                E.tensor_add(G01, q1, q2)
                E.tensor_mul(q1, g10, w00)
                E.tensor_mul(q2, g11, w01)
                E.tensor_add(G10, q1, q2)
                E.tensor_mul(q1, g10, w01)
                E.tensor_mul(q2, g11, w11)
                E.tensor_add(G11, q1, q2)
                E.tensor_mul(q1, G00, MR)
                E.tensor_mul(q2, G01, MI)
                E.tensor_add(q1, q1, q2)
                E.tensor_sub(BR, br_, q1)
                E.tensor_mul(q1, G10, MR)
                E.tensor_mul(q2, G11, MI)
                E.tensor_add(q1, q1, q2)
                E.tensor_sub(BI, bi_, q1)
                # Per-engine staged copies: whiten stage-1 (ACT) and stage-2
                # (DVE) read scale/bias scalars from tiles produced on their
                # own engine, so program order guarantees readiness.
                cbA = smallp.tile([P, 6], f32, tag="cbA")
                A.copy(cbA[:], cb[:])
                cbD = smallp.tile([P, 6], f32, tag="cbD")
                V.tensor_copy(cbD[:], cb[:])
                state[c]["cb"] = cb
                state[c]["cbA"] = cbA
                state[c]["cbD"] = cbD

            def emit_whiten(c):
                # ---- whiten + affine + store, per half-quarter block ----
                xq = state[c]["xq"]
                cbA, cbD = state[c]["cbA"], state[c]["cbD"]
                for h in range(2 * NQ):
                    q, hh = h // 2, h % 2
                    x3 = xq[q][:].bitcast(f32).rearrange(
                        "p (j k) -> p j k", k=BLK)
                    xr = x3[:, hh * HCH:(hh + 1) * HCH, 0:CHUNK]
                    xi = x3[:, hh * HCH:(hh + 1) * HCH, CHUNK:2 * CHUNK]
                    yt = yp.tile([P, HCH, 2 * CHUNK], f32, tag="y")
                    ye = yt[:, :, 0:2 * CHUNK:2]
                    yo = yt[:, :, 1:2 * CHUNK:2]
                    A.activation(ye, xr, Act.Identity,
                                 bias=cbA[:, 2:3], scale=cbA[:, 0:1])
                    A.activation(yo, xr, Act.Identity,
                                 bias=cbA[:, 5:6], scale=cbA[:, 3:4])
                    V.scalar_tensor_tensor(out=ye, in0=xi,
                                           scalar=cbD[:, 1:2],
                                           in1=ye, op0=Alu.mult, op1=Alu.add)
                    V.scalar_tensor_tensor(out=yo, in0=xi,
                                           scalar=cbD[:, 4:5],
                                           in1=yo, op0=Alu.mult, op1=Alu.add)
                    # Late channels run after the last load has issued, so
                    # the idle SP queue takes every other store; Pool's
                    # SWDGE generation alone would pace the drain.
                    eng = nc.sync if (c >= CLOC - 3 and h % 2 == 1) else GS
                    eng.dma_start(
                        out=y_d[c][:, h * HOUT:(h + 1) * HOUT],
                        in_=yt[:].rearrange("p a b -> p (a b)"))

            # Lag-2 software pipeline. Per iteration i the engine streams see:
            #   SP:   loads(i)
            #   PE:   gram(i) [folds(i-2) slotted in after quarter 0]
            #   DVE:  extract(i-1), asm(i-2), whiten-stage2(i-2)
            #   ACT:  colsum-copies(i-1), sqrts/cbA(i-2), whiten-stage1(i-2)
            #   Pool: sp-memsets(i-1), stores(i-2)
            # Every queue head's semaphore wait targets work from >= one
            # channel-period earlier, so no engine ever idles with ready
            # work parked behind a blocked instruction.
            for i in range(CLOC + 2):
                if i < CLOC:
                    state[i] = {}
                    state[i]["xq"] = emit_loads(i)
                    state[i]["g"] = emit_gram(i, state[i]["xq"])
                else:
                    emit_folds(i - 2)
                if 1 <= i <= CLOC:
                    state[i - 1]["sp"] = emit_extract(i - 1,
                                                      state[i - 1]["g"])
                if i >= 2:
                    emit_asm(i - 2)
                    emit_whiten(i - 2)

    nc.finalize()
    return nc


def _get_nc():
    if "nc" not in _CACHE:
        _CACHE["nc"] = _build_nc()
    return _CACHE["nc"]


def _prep_core(x_real, x_imag, gamma, beta, k):
    c0 = k * CLOC
    xr = np.ascontiguousarray(
        x_real[:, c0:c0 + CLOC].transpose(1, 0, 2, 3)
    ).reshape(CLOC, P, NCHUNK, CHUNK)
    xi = np.ascontiguousarray(
        x_imag[:, c0:c0 + CLOC].transpose(1, 0, 2, 3)
    ).reshape(CLOC, P, NCHUNK, CHUNK)
    xy = np.empty((CLOC, P, NCHUNK, BLK), np.float32)
    xy[..., 0:CHUNK] = xr
    xy[..., CHUNK:2 * CHUNK] = xi
    g = gamma[c0:c0 + CLOC]
    b = beta[c0:c0 + CLOC]
    gb = np.concatenate([g[:, 0, 0], g[:, 0, 1], g[:, 1, 0], g[:, 1, 1],
                         b[:, 0], b[:, 1]]).astype(np.float32).reshape(1, 48)
    return {"xy": xy.reshape(CLOC, P, XYW), "gb": gb}


def kernel(x_real, x_imag, gamma, beta):
    from concourse.bass_utils import run_bass_kernel_spmd

    x_real = np.asarray(x_real, dtype=np.float32)
    x_imag = np.asarray(x_imag, dtype=np.float32)
    gamma = np.asarray(gamma, dtype=np.float32)
    beta = np.asarray(beta, dtype=np.float32)

    in_maps = [_prep_core(x_real, x_imag, gamma, beta, k)
               for k in range(NCORES)]

    nc = _get_nc()
    res = None
    if _TRACE:
        try:
            res = run_bass_kernel_spmd(nc, in_maps, list(range(NCORES)),
                                       trace=True)
        except Exception as e:  # trace infra unavailable -> plain run
            LAST["trace_error"] = repr(e)
            res = None
    if res is None:
        res = run_bass_kernel_spmd(nc, in_maps, list(range(NCORES)))
    LAST["exec_time_ns"] = res.exec_time_ns
    LAST["mean_exec_time_ns"] = res.mean_exec_time_ns
    LAST["profile_json"] = res.profile_json

    out = np.empty((B, C, H, W, 2), np.float32)
    for k in range(NCORES):
        c0 = k * CLOC
        y = res.results[k]["y"].reshape(CLOC, N, 2).reshape(CLOC, B, H, W, 2)
        out[:, c0:c0 + CLOC] = y.transpose(1, 0, 2, 3, 4)
    return out
